# revision 31
# baseline (speedup 1.0000x reference)
"""Trainium2 Bass kernel for 2-layer GAT (nn_FAGAT) over 8 NeuronCores.

v2 design (node/dst-sharded, compact-gather message passing, bf16):
  - Core c owns dst nodes [c*SHARD, (c+1)*SHARD). Self-loop edges are handled
    densely (per dst block, S = identity); only real edges go through the
    gather pipeline.
  - Per core, the unique src set (~29k < 32768) indexes a COMPACT table, so
    one int16 index stream covers both layers:
      * L1: host ships x rows (bf16, 128 cols = 256B) at compact slots;
        per window of 16 chunks a single transpose=True dma_gather yields
        xt [feat, edge] directly (no PE transpose, no PSUM evacuation).
      * L2: after the AllGather each core re-compacts the global h2 table
        (two range-gathers, lo/hi of the sorted unique list) into a local
        compact DRAM table; edge gathers then reuse the SAME idx arrays.
  - One-hot S (bf16, DVE is_equal) turns segment softmax + weighted scatter
    into PSUM-accumulated matmuls; ST = PE-transposed S expands per-dst
    attention halves to edges.
  - Per-window batching: attention logits for 16 chunks are accumulated into
    one PSUM tile by tiny matmuls, then ONE leaky-relu + ONE exp serve the
    whole window.  PSUM evacuations ride the Activation engine.
  - Everything hot is bf16 (DVE 2x/4x modes, PE 1 cycle/row); accumulations
    stay f32 in PSUM.
"""
import os
os.environ.setdefault("NEURON_SCRATCHPAD_PAGE_SIZE", "64")
import sys
if "/opt/trn_rl_repo" not in sys.path:
    sys.path.insert(0, "/opt/trn_rl_repo")

from dataclasses import dataclass, field
import numpy as np

import concourse.bass as bass
import concourse.mybir as mybir
from concourse import bacc, tile
from concourse.bass_utils import run_bass_kernel_spmd

F32 = mybir.dt.float32
BF16 = mybir.dt.bfloat16
I16 = mybir.dt.int16
AF = mybir.ActivationFunctionType
OP = mybir.AluOpType

NEG = 0.2
EPS = 1e-16


def to_bf16(a):
    import ml_dtypes
    return np.asarray(a, dtype=np.float32).astype(ml_dtypes.bfloat16)


@dataclass
class Cfg:
    N: int = 50000
    NC: int = 8
    SPLIT: int = 32768
    KIN: int = 27
    K1: int = 32           # padded input features
    H1: int = 4
    D1: int = 64
    H2: int = 2
    D2: int = 64
    WCH: int = 16          # chunks per gather window
    GRP: int = 8           # blocks per normalization group
    CTC: int = 256         # cc table cols (bf16, 512B rows)
    timing_single_core: bool = False
    stop_after: int = 99   # debug: 1=dense, 2=L1, 3=h2, 4=gather/copy, 5=compaction

    @property
    def SHARD(self):
        return self.N // self.NC

    @property
    def NBLK(self):
        return (self.SHARD + 127) // 128

    @property
    def F1(self):
        return self.H1 * self.D1   # 256

    @property
    def F2(self):
        return self.H2 * self.D2   # 128


@dataclass
class Structure:
    nch: np.ndarray = None      # [NBLK] chunks per block (shared both layers)
    chunks: list = field(default_factory=list)  # (block, slot, first, last)
    wblocks: dict = field(default_factory=dict)  # w -> [block of each wslot]
    cores: list = field(default_factory=list)
    NCH: int = 0
    KLOC: int = 0               # compaction lo chunks (global max)
    KHIC: int = 0
    CC: int = 0                 # KLOC + KHIC


def wrap_idx(a, nch):
    """int16 idx array [nch*128] -> [128, nch*8] wrapped gather layout."""
    w = a.astype(np.int16).reshape(nch * 8, 16).T
    return np.tile(w, (8, 1)).copy()


def prep_edges(cfg: Cfg, src, dst):
    NC, SHARD, NBLK = cfg.NC, cfg.SHARD, cfg.NBLK
    src = np.asarray(src, dtype=np.int64)
    dst = np.asarray(dst, dtype=np.int64)
    per_core = []
    for c in range(NC):
        m = (dst // SHARD) == c
        es, ed = src[m], dst[m] - c * SHARD
        uniq = np.unique(es)
        assert len(uniq) < 32768, f"core {c}: {len(uniq)} unique srcs"
        cpos = np.searchsorted(uniq, es)
        k = int(np.searchsorted(uniq, cfg.SPLIT))
        blocks = []
        for b in range(NBLK):
            bm = (ed // 128) == b
            blocks.append((cpos[bm], ed[bm] - b * 128))
        per_core.append(dict(uniq=uniq, k=k, blocks=blocks))

    nch = np.zeros(NBLK, dtype=int)
    for c in range(NC):
        for b in range(NBLK):
            nch[b] = max(nch[b], -(-len(per_core[c]["blocks"][b][0]) // 128))

    st = Structure(nch=nch)
    slot = 0
    for b in range(NBLK):
        n = int(nch[b])
        for kk in range(n):
            st.chunks.append((b, slot, kk == 0, kk == n - 1))
            w, wi = divmod(slot, cfg.WCH)
            st.wblocks.setdefault(w, {})[wi] = b
            slot += 1
    st.NCH = slot

    st.KLOC = max(-(-pc["k"] // 128) for pc in per_core)
    st.KHIC = max(-(-(len(pc["uniq"]) - pc["k"]) // 128) for pc in per_core)
    st.CC = st.KLOC + st.KHIC

    NCHp = ((st.NCH + cfg.WCH - 1) // cfg.WCH) * cfg.WCH
    for c in range(NC):
        pc = per_core[c]
        uniq, k = pc["uniq"], pc["k"]
        # compact slot of unique position j
        def slot_of(j):
            return np.where(j < k, j, st.KLOC * 128 + (j - k))
        idx1 = np.zeros(NCHp * 128, np.int32)
        dl = np.full(NCHp * 128, -1.0, np.float32)
        o = 0
        for b in range(NBLK):
            cp, dloc = pc["blocks"][b]
            nb = int(nch[b])
            idx1[o:o + len(cp)] = slot_of(cp)
            dl[o:o + len(cp)] = dloc
            o += nb * 128
        # compaction gather ids: lo section then hi section (padded w/ 0)
        idxc = np.zeros(st.CC * 128, np.int32)
        idxc[0:k] = uniq[0:k]
        idxc[st.KLOC * 128:st.KLOC * 128 + (len(uniq) - k)] = uniq[k:] - cfg.SPLIT
        # compact x table (bf16 rows at compact slots)
        xrow = np.zeros((st.CC * 128, cfg.K1 * 4), np.float32)  # 128 cols
        st.cores.append(dict(
            idx1=wrap_idx(idx1, NCHp),
            dl=dl.reshape(NCHp, 128).T.copy(),     # [128, NCHp]
            idxc=wrap_idx(idxc, st.CC),
            _uniq=uniq, _k=k, _xrow=xrow,
        ))
    st.NCHp = NCHp
    return st


def host_inputs(cfg: Cfg, st: Structure, inputs):
    x = np.asarray(inputs["x"], dtype=np.float32)
    W1 = np.asarray(inputs["W1"], np.float32)
    a_src1 = np.asarray(inputs["a_src1"], np.float32)
    a_dst1 = np.asarray(inputs["a_dst1"], np.float32)
    W2 = np.asarray(inputs["W2"], np.float32)
    a_src2 = np.asarray(inputs["a_src2"], np.float32)
    a_dst2 = np.asarray(inputs["a_dst2"], np.float32)

    # W1E [K1, F1]; A1SD [K1, 8]: cols 0:4 = W1@a_dst1, 4:8 = W1@a_src1
    W1E = np.zeros((cfg.K1, cfg.F1), np.float32)
    W1E[:cfg.KIN] = W1
    A1SD = np.zeros((cfg.K1, 2 * cfg.H1), np.float32)
    for h in range(cfg.H1):
        A1SD[:cfg.KIN, h] = W1[:, h * cfg.D1:(h + 1) * cfg.D1] @ a_dst1[h]
        A1SD[:cfg.KIN, cfg.H1 + h] = W1[:, h * cfg.D1:(h + 1) * cfg.D1] @ a_src1[h]
    # W2F [F1, 132] = [W2 | W2@a_src2 | W2@a_dst2], k-tiled to [128, 2, 132]
    W2F = np.zeros((cfg.F1, cfg.F2 + 2 * cfg.H2), np.float32)
    W2F[:, :cfg.F2] = W2
    for h in range(cfg.H2):
        W2F[:, cfg.F2 + h] = W2[:, h * cfg.D2:(h + 1) * cfg.D2] @ a_src2[h]
        W2F[:, cfg.F2 + cfg.H2 + h] = W2[:, h * cfg.D2:(h + 1) * cfg.D2] @ a_dst2[h]
    W2F = np.ascontiguousarray(
        W2F.reshape(2, 128, cfg.F2 + 2 * cfg.H2).transpose(1, 0, 2))

    iota = np.tile(np.arange(128, dtype=np.float32), (128, 1))
    ident = np.eye(128, dtype=np.float32)
    wfcrow = np.tile(np.asarray(inputs["Wfc"], np.float32).reshape(1, -1), (128, 1))
    bfccol = np.full((128, 1), np.asarray(inputs["bfc"], np.float32).reshape(-1)[0],
                     dtype=np.float32)
    assert not np.any(np.asarray(inputs["b1"])) and \
        not np.any(np.asarray(inputs["b2"])), "nonzero biases unsupported"

    shared = dict(W1E=W1E, A1SD=A1SD, W2F=W2F, IOTA=iota, IDENT=ident,
                  WFCROW=wfcrow, BFCC=bfccol)

    in_maps = []
    for c in range(cfg.NC):
        m = dict(shared)
        pc = st.cores[c]
        uniq, k = pc["_uniq"], pc["_k"]
        # compact x table: rows at gapped compact slots, bf16
        xtab = np.zeros((st.CC * 128, 128), np.float32)
        xtab[0:k, :cfg.KIN] = x[uniq[0:k]]
        xtab[st.KLOC * 128:st.KLOC * 128 + len(uniq) - k, :cfg.KIN] = x[uniq[k:]]
        m["XCTAB"] = to_bf16(xtab)
        # own-shard x transposed [K1, NBLK*128]
        xtd = np.zeros((cfg.K1, cfg.NBLK * 128), np.float32)
        nrow = min(cfg.SHARD, cfg.N - c * cfg.SHARD)
        xtd[:cfg.KIN, :nrow] = x[c * cfg.SHARD:c * cfg.SHARD + nrow].T
        m["XTD"] = xtd
        m["idx1"] = pc["idx1"]
        m["idxc"] = pc["idxc"]
        # host-built one-hot S / ST, window-major layout [NW*128, WCH*128]
        dl = pc["dl"]                                   # [128, NCHp]
        NCHp = dl.shape[1]
        NW = NCHp // cfg.WCH
        S_full = (dl[:, :, None] ==
                  np.arange(128, dtype=np.float32)[None, None, :])
        Sw = S_full.reshape(128, NW, cfg.WCH, 128).transpose(1, 0, 2, 3)
        m["SH"] = to_bf16(Sw.reshape(NW * 128, cfg.WCH * 128))
        STw = S_full.transpose(2, 1, 0).reshape(128, NW, cfg.WCH, 128) \
            .transpose(1, 0, 2, 3)
        m["STH"] = to_bf16(STw.reshape(NW * 128, cfg.WCH * 128))
        in_maps.append(m)
    return in_maps


# --------------------------------------------------------------------------
#  device program
# --------------------------------------------------------------------------

def emit_gat(tc, outs, ins, cfg: Cfg, st: Structure):
    nc = tc.nc
    NBLK, F1, F2, H1, H2 = cfg.NBLK, cfg.F1, cfg.F2, cfg.H1, cfg.H2
    WCH, GRP, CTC = cfg.WCH, cfg.GRP, cfg.CTC
    NCH = st.NCH
    NW = (NCH + WCH - 1) // WCH
    y = outs["y"]

    cc_in = nc.dram_tensor("cc_in", [cfg.SHARD, CTC], BF16, kind="Internal").ap()
    cc_out = nc.dram_tensor("cc_out", [cfg.N, CTC], BF16, kind="Internal",
                            addr_space="Shared").ap()
    ctab = nc.dram_tensor("ctab", [st.CC * 128, CTC], BF16, kind="Internal").ap()

    with (
        tc.tile_pool(name="const", bufs=1) as constp,
        tc.tile_pool(name="big", bufs=1) as bigp,
    ):
        def cload(name, dtype=F32):
            src = ins[name]
            t = constp.tile(list(src.shape), dtype, tag=name)
            nc.sync.dma_start(t[:], src)
            return t

        def cload_bf(name):
            f = cload(name)
            t = constp.tile(list(ins[name].shape), BF16, tag=name + "b")
            nc.vector.tensor_copy(t[:], f[:])
            return t

        IDENT = cload_bf("IDENT")
        W1E = cload_bf("W1E")
        A1SD = cload_bf("A1SD")
        W2F = cload_bf("W2F")
        WFC = cload_bf("WFCROW")
        BFCC = cload("BFCC")
        XTD = cload_bf("XTD")
        IDX1 = cload("idx1", dtype=I16)
        IDXC = cload("idxc", dtype=I16)

        x2_all = bigp.tile([128, NBLK, F1], BF16)
        h2_sb = bigp.tile([128, NBLK, F2 + 2 * H2], BF16)
        sdss = bigp.tile([128, NBLK, 2 * H1], BF16)   # [sdst1 | ssrc1_own]
        wself = bigp.tile([128, NBLK, H1], BF16)
        w2self = bigp.tile([128, NBLK, H2], BF16)

        # ---------------- dense phase: sdst1/ssrc1_own + self weights ------
        with (
            tc.tile_pool(name="dn", bufs=1) as dnp,
            tc.tile_pool(name="ps_dn", bufs=1, space="PSUM") as psdn,
        ):
            sd_ps = psdn.tile([128, NBLK, 2 * H1], F32)
            for b in range(NBLK):
                nc.tensor.matmul(sd_ps[:, b, :], XTD[:, b * 128:(b + 1) * 128],
                                 A1SD[:], start=(b == 0), stop=(b == NBLK - 1),
                                 skip_group_check=True)
            nc.scalar.copy(sdss[:], sd_ps[:])
            tself = dnp.tile([128, NBLK, H1], BF16)
            nc.vector.tensor_tensor(tself[:], sdss[:, :, 0:H1],
                                    sdss[:, :, H1:2 * H1], OP.add)
            lr = dnp.tile([128, NBLK, H1], BF16)
            nc.vector.scalar_tensor_tensor(lr[:], tself[:], NEG, tself[:],
                                           OP.mult, OP.max)
            nc.scalar.activation(wself[:], lr[:], AF.Exp)
        if cfg.stop_after <= 1:
            nc.gpsimd.dma_start(y[0:128, :], wself[0:128, 0, 0:1])
            return

        # ---------------- layer 1 ----------------
        with (
            tc.tile_pool(name="l1g", bufs=3) as gpool,
            tc.tile_pool(name="l1s", bufs=3) as spool,
            tc.tile_pool(name="l1w", bufs=3) as wpool,
            tc.tile_pool(name="l1gw", bufs=10) as gwpool,
            tc.tile_pool(name="l1ng", bufs=2) as ngp,
            tc.tile_pool(name="l1ev", bufs=1) as evp,
            tc.tile_pool(name="ps_hs", bufs=4, space="PSUM") as pshs,
            tc.tile_pool(name="ps_ss", bufs=2, space="PSUM") as psss,
            tc.tile_pool(name="ps_blk", bufs=2, space="PSUM") as psblk,
        ):
            windows = {}

            def get_window(w):
                if w in windows:
                    return windows[w]
                n = min(WCH, NCH - w * WCH)
                xtw = gpool.tile([128, 1, WCH * 128], BF16, tag="xtw")
                # transposed gathers crash above 512 idxs -> 4-chunk pieces
                for q0 in range(0, n, 4):
                    qn = min(4, n - q0)
                    nc.gpsimd.dma_gather(
                        xtw[:, :, q0 * 128:(q0 + qn) * 128], ins["XCTAB"],
                        IDX1[:, (w * WCH + q0) * 8:(w * WCH + q0 + qn) * 8],
                        qn * 128, qn * 128, 128, transpose=True)
                Sb = spool.tile([128, WCH, 128], BF16, tag="Sb")
                nc.sync.dma_start(
                    Sb[:].rearrange("p c j -> p (c j)")[:, 0:n * 128],
                    ins["SH"][w * 128:(w + 1) * 128, 0:n * 128])
                STb = spool.tile([128, WCH, 128], BF16, tag="STb")
                nc.sync.dma_start(
                    STb[:].rearrange("p c j -> p (c j)")[:, 0:n * 128],
                    ins["STH"][w * 128:(w + 1) * 128, 0:n * 128])
                # window logits: ssrc (tiny matmul) + sdst expand, batched
                ss_ps = psss.tile([128, WCH, H1], F32, tag="ssps")
                for ci in range(n):
                    bb = st.wblocks[w][ci]
                    nc.tensor.matmul(ss_ps[:, ci, :],
                                     xtw[0:cfg.K1, 0, ci * 128:(ci + 1) * 128],
                                     A1SD[:, H1:2 * H1],
                                     start=(ci == 0), stop=False,
                                     skip_group_check=True)
                    nc.tensor.matmul(ss_ps[:, ci, :], STb[:, ci, :],
                                     sdss[:, bb, 0:H1],
                                     start=False, stop=(ci == n - 1),
                                     skip_group_check=True)
                ss_sb = wpool.tile([128, WCH, H1], F32, tag="sssb")
                nc.scalar.copy(ss_sb[:, 0:n, :], ss_ps[:, 0:n, :])
                lr = wpool.tile([128, WCH, H1], F32, tag="lr")
                nc.vector.scalar_tensor_tensor(lr[:, 0:n, :], ss_sb[:, 0:n, :],
                                               NEG, ss_sb[:, 0:n, :],
                                               OP.mult, OP.max)
                wv = wpool.tile([128, WCH, H1], BF16, tag="wv")
                nc.scalar.activation(wv[:, 0:n, :], lr[:, 0:n, :], AF.Exp)
                windows[w] = (xtw, Sb, wv)
                return windows[w]

            # item stream: per block a dense self pseudo-chunk then edge chunks
            items = []
            slot = 0
            for b in range(NBLK):
                nb = int(st.nch[b])
                items.append(("self", b, None, nb == 0))
                for kk in range(nb):
                    items.append(("edge", b, slot, kk == nb - 1))
                    slot += 1
            NIT = len(items)

            hs_ap = [None] * NIT
            gw_ap = [None] * NIT
            blk_of = {}
            ng_of = {}
            hs_bank = [None]

            def emit_hs(i):
                kind, b, sl, last = items[i]
                if kind == "edge":
                    get_window(sl // WCH)
                if i % 2 == 0:
                    hs_bank[0] = pshs.tile([128, 2 * F1], F32, tag="hs",
                                           name="hsbank")
                hs = hs_bank[0][:, (i % 2) * F1:(i % 2 + 1) * F1]
                if kind == "self":
                    lhsT = XTD[:, b * 128:(b + 1) * 128]
                else:
                    xtw = windows[sl // WCH][0]
                    wi = sl % WCH
                    lhsT = xtw[0:cfg.K1, 0, wi * 128:(wi + 1) * 128]
                nc.tensor.matmul(hs, lhsT, W1E[:], start=(i % 2 == 0),
                                 stop=(i % 2 == 1) or i == NIT - 1,
                                 skip_group_check=True)
                hs_ap[i] = hs

            def emit_gw(i):
                kind, b, sl, last = items[i]
                if kind == "self":
                    wvv = wself[:, b, :]
                else:
                    w, wi = divmod(sl, WCH)
                    wvv = windows[w][2][:, wi, :]
                gw = gwpool.tile([128, F1], BF16, tag="gw")
                if i % 2 == 0:
                    src = hs_ap[i]
                else:
                    src = gwpool.tile([128, F1], BF16, tag="hsbf")
                    nc.scalar.copy(src[:], hs_ap[i])
                    src = src[:]
                nc.vector.tensor_tensor(
                    gw[:].rearrange("p (h d) -> p h d", d=cfg.D1),
                    src.rearrange("p (h d) -> p h d", d=cfg.D1),
                    wvv.rearrange("p (h u) -> p h u", u=1)
                        .to_broadcast((128, H1, cfg.D1)),
                    OP.mult)
                gw_ap[i] = gw

            def emit_scatter(i):
                kind, b, sl, last = items[i]
                if kind == "self":
                    blk_ps = psblk.tile([128, F1 + H1], F32, tag="blk")
                    blk_of[b] = blk_ps
                    S = IDENT[:]
                    wvv = wself[:, b, :]
                    first = True
                else:
                    blk_ps = blk_of[b]
                    w, wi = divmod(sl, WCH)
                    S = windows[w][1][:, wi, :]
                    wvv = windows[w][2][:, wi, :]
                    first = False
                nc.tensor.matmul(blk_ps[:, 0:F1], S, gw_ap[i][:],
                                 start=first, stop=last,
                                 skip_group_check=True)
                nc.tensor.matmul(blk_ps[:, F1:F1 + H1], S, wvv,
                                 start=False, stop=last,
                                 skip_group_check=True)
                hs_ap[i] = None
                gw_ap[i] = None
                if last:
                    stash(b, blk_ps)
                    del blk_of[b]

            def stash(b, blk_ps):
                if b % GRP == 0:
                    ng_of["num"] = ngp.tile([128, GRP, F1], BF16, tag="numbf", name="numbf")
                    ng_of["den"] = ngp.tile([128, GRP, H1], F32, tag="denf", name="denf")
                num_bf, den_f = ng_of["num"], ng_of["den"]
                nc.scalar.copy(num_bf[:, b % GRP, :], blk_ps[:, 0:F1])
                nc.vector.tensor_copy(den_f[:, b % GRP, :],
                                      blk_ps[:, F1:F1 + H1])
                if b % GRP == GRP - 1 or b == NBLK - 1:
                    g0 = (b // GRP) * GRP
                    gn = b - g0 + 1
                    rd = evp.tile([128, GRP, H1], F32, tag="rd")
                    nc.vector.reciprocal(rd[:, 0:gn, :], den_f[:, 0:gn, :])
                    rdb = evp.tile([128, GRP, H1], BF16, tag="rdb")
                    nc.vector.tensor_copy(rdb[:, 0:gn, :], rd[:, 0:gn, :])
                    xg = evp.tile([128, GRP, F1], BF16, tag="xg")
                    nc.vector.tensor_tensor(
                        xg[:, 0:gn, :].rearrange("p g (h d) -> p g h d", d=cfg.D1),
                        num_bf[:, 0:gn, :].rearrange("p g (h d) -> p g h d",
                                                     d=cfg.D1),
                        rdb[:, 0:gn, :].rearrange("p g (h u) -> p g h u", u=1)
                            .to_broadcast((128, gn, H1, cfg.D1)),
                        OP.mult)
                    # elu: exp(min(x,0)) - 1 + max(x,0)
                    tm = evp.tile([128, GRP, F1], BF16, tag="tm")
                    nc.vector.tensor_scalar(tm[:, 0:gn, :], xg[:, 0:gn, :],
                                            0.0, None, OP.min)
                    te = evp.tile([128, GRP, F1], BF16, tag="te")
                    nc.scalar.activation(te[:, 0:gn, :], tm[:, 0:gn, :], AF.Exp)
                    nc.vector.tensor_scalar(tm[:, 0:gn, :], xg[:, 0:gn, :],
                                            0.0, -1.0, OP.max, OP.add)
                    nc.vector.tensor_tensor(x2_all[:, g0:g0 + gn, :],
                                            te[:, 0:gn, :], tm[:, 0:gn, :],
                                            OP.add)

            PIPE_G = 4
            prev = None
            for i0 in range(0, NIT, PIPE_G):
                grp = list(range(i0, min(i0 + PIPE_G, NIT)))
                for i in grp:
                    emit_hs(i)
                if prev:
                    for i in prev:
                        emit_gw(i)
                    for i in prev:
                        emit_scatter(i)
                prev = grp
            for i in prev:
                emit_gw(i)
            for i in prev:
                emit_scatter(i)

        if cfg.stop_after <= 2:
            nc.gpsimd.dma_start(y[0:128, :], x2_all[0:128, 0, 0:1])
            return

        # ---------------- h2 build + AllGather + compaction ----------------
        F2E = F2 + 2 * H2
        with (
            tc.tile_pool(name="h2sb", bufs=2) as hsb,
            tc.tile_pool(name="ps_h2", bufs=2, space="PSUM") as psh,
            tc.tile_pool(name="ps_h2t", bufs=2, space="PSUM") as psht,
        ):
            for b in range(NBLK):
                rows = min(128, cfg.SHARD - b * 128)
                xt2_ps = psht.tile([128, 2, 128], BF16, tag="x2t")
                for k in range(2):
                    nc.tensor.transpose(xt2_ps[:, k, :],
                                        x2_all[:, b, k * 128:(k + 1) * 128],
                                        IDENT[:])
                xt2 = hsb.tile([128, 2, 128], BF16, tag="x2sb")
                nc.vector.tensor_copy(xt2[:], xt2_ps[:])
                h2_ps = psh.tile([128, F2E], F32, tag="h2ps")
                for k in range(2):
                    nc.tensor.matmul(h2_ps[:], xt2[:, k, :], W2F[:, k, :],
                                     start=(k == 0), stop=(k == 1),
                                     skip_group_check=True)
                nc.scalar.copy(h2_sb[:, b, :], h2_ps[:])
                nc.sync.dma_start(cc_in[b * 128:b * 128 + rows, 0:F2 + H2],
                                  h2_sb[0:rows, b, 0:F2 + H2])
            # self weights for layer 2
            t2 = hsb.tile([128, NBLK, H2], BF16, tag="t2")
            nc.vector.tensor_tensor(t2[:], h2_sb[:, :, F2:F2 + H2],
                                    h2_sb[:, :, F2 + H2:F2E], OP.add)
            lr2 = hsb.tile([128, NBLK, H2], BF16, tag="lr2")
            nc.vector.scalar_tensor_tensor(lr2[:], t2[:], NEG, t2[:],
                                           OP.mult, OP.max)
            nc.scalar.activation(w2self[:], lr2[:], AF.Exp)

        if cfg.stop_after <= 3:
            nc.gpsimd.dma_start(y[0:128, :], h2_sb[0:128, 0, 0:1])
            return

        if cfg.timing_single_core:
            nc.sync.dma_start(cc_out[0:cfg.SHARD, :], cc_in[:])
        else:
            nc.gpsimd.collective_compute(
                "AllGather", OP.bypass,
                replica_groups=[list(range(cfg.NC))],
                ins=[cc_in[:]],
                outs=[cc_out[:]],
            )

        if cfg.stop_after <= 4:
            nc.gpsimd.dma_start(y[0:128, :], h2_sb[0:128, 0, 0:1])
            return

        # compaction: gather unique rows from cc_out into local compact ctab
        with tc.tile_pool(name="cg", bufs=4) as cgp:
            ctv = ctab.rearrange("(c p) f -> p c f", p=128)
            F2U = F2 + H2    # used cols of a cc row
            for sec, base, ncc in ((0, 0, st.KLOC), (1, st.KLOC, st.KHIC)):
                tab = cc_out[0:cfg.SPLIT, :] if sec == 0 else \
                    cc_out[cfg.SPLIT:cfg.N, :]
                for w0 in range(0, ncc, WCH):
                    n = min(WCH, ncc - w0)
                    gt = cgp.tile([128, WCH, CTC], BF16, tag="cgt")
                    for q0 in range(0, n, 8):  # <=1024 idxs per gather
                        qn = min(8, n - q0)
                        nc.gpsimd.dma_gather(
                            gt[:, q0:q0 + qn, :], tab,
                            IDXC[:, (base + w0 + q0) * 8:
                                 (base + w0 + q0 + qn) * 8],
                            qn * 128, qn * 128, CTC)
                    nc.sync.dma_start(
                        ctv[:, base + w0:base + w0 + n, 0:F2U],
                        gt[:, 0:n, 0:F2U])

        if cfg.stop_after <= 5:
            nc.gpsimd.dma_start(y[0:128, :], h2_sb[0:128, 0, 0:1])
            return

        # ---------------- layer 2 ----------------
        with (
            tc.tile_pool(name="l2g", bufs=3) as gpool,
            tc.tile_pool(name="l2s", bufs=3) as spool,
            tc.tile_pool(name="l2w", bufs=3) as wpool,
            tc.tile_pool(name="l2gw", bufs=10) as gwpool,
            tc.tile_pool(name="l2ng", bufs=2) as ngp,
            tc.tile_pool(name="l2ev", bufs=1) as evp,
            tc.tile_pool(name="ps_ss2", bufs=3, space="PSUM") as psss,
            tc.tile_pool(name="ps_blk2", bufs=2, space="PSUM") as psblk,
        ):
            windows = {}

            def get_window2(w):
                if w in windows:
                    return windows[w]
                n = min(WCH, NCH - w * WCH)
                gt = gpool.tile([128, WCH, CTC], BF16, tag="gt")
                for q0 in range(0, n, 8):  # <=1024 idxs per gather
                    qn = min(8, n - q0)
                    nc.gpsimd.dma_gather(
                        gt[:, q0:q0 + qn, :], ctab,
                        IDX1[:, (w * WCH + q0) * 8:(w * WCH + q0 + qn) * 8],
                        qn * 128, qn * 128, CTC)
                Sb = spool.tile([128, WCH, 128], BF16, tag="Sb2")
                nc.sync.dma_start(
                    Sb[:].rearrange("p c j -> p (c j)")[:, 0:n * 128],
                    ins["SH"][w * 128:(w + 1) * 128, 0:n * 128])
                STb = spool.tile([128, WCH, 128], BF16, tag="STb2")
                nc.sync.dma_start(
                    STb[:].rearrange("p c j -> p (c j)")[:, 0:n * 128],
                    ins["STH"][w * 128:(w + 1) * 128, 0:n * 128])
                ss_ps = psss.tile([128, WCH, H2], F32, tag="ssps2")
                for ci in range(n):
                    bb = st.wblocks[w][ci]
                    nc.tensor.matmul(ss_ps[:, ci, :], STb[:, ci, :],
                                     h2_sb[:, bb, F2 + H2:F2E],
                                     start=(ci == 0), stop=(ci == n - 1),
                                     skip_group_check=True)
                t = wpool.tile([128, WCH, H2], F32, tag="t")
                nc.vector.tensor_tensor(t[:, 0:n, :], ss_ps[:, 0:n, :],
                                        gt[:, 0:n, F2:F2 + H2], OP.add)
                lr = wpool.tile([128, WCH, H2], F32, tag="lr")
                nc.vector.scalar_tensor_tensor(lr[:, 0:n, :], t[:, 0:n, :],
                                               NEG, t[:, 0:n, :],
                                               OP.mult, OP.max)
                wv = wpool.tile([128, WCH, H2], BF16, tag="wv")
                nc.scalar.activation(wv[:, 0:n, :], lr[:, 0:n, :], AF.Exp)
                windows[w] = (gt, Sb, wv)
                return windows[w]

            items = []
            slot = 0
            for b in range(NBLK):
                nb = int(st.nch[b])
                items.append(("self", b, None, nb == 0))
                for kk in range(nb):
                    items.append(("edge", b, slot, kk == nb - 1))
                    slot += 1
            NIT = len(items)
            gw_ap = [None] * NIT
            blk_of = {}
            ng_of = {}

            def emit_gw2(i):
                kind, b, sl, last = items[i]
                gw = gwpool.tile([128, F2], BF16, tag="gw2")
                if kind == "self":
                    src = h2_sb[:, b, 0:F2]
                    wvv = w2self[:, b, :]
                else:
                    w, wi = divmod(sl, WCH)
                    gt, _, wv = get_window2(w)
                    src = gt[:, wi, 0:F2]
                    wvv = wv[:, wi, :]
                nc.vector.tensor_tensor(
                    gw[:].rearrange("p (h d) -> p h d", d=cfg.D2),
                    src.rearrange("p (h d) -> p h d", d=cfg.D2),
                    wvv.rearrange("p (h u) -> p h u", u=1)
                        .to_broadcast((128, H2, cfg.D2)),
                    OP.mult)
                gw_ap[i] = gw

            def emit_scatter2(i):
                kind, b, sl, last = items[i]
                if kind == "self":
                    blk_ps = psblk.tile([128, F2 + H2], F32, tag="blk2")
                    blk_of[b] = blk_ps
                    S = IDENT[:]
                    wvv = w2self[:, b, :]
                    first = True
                else:
                    blk_ps = blk_of[b]
                    w, wi = divmod(sl, WCH)
                    S = windows[w][1][:, wi, :]
                    wvv = windows[w][2][:, wi, :]
                    first = False
                nc.tensor.matmul(blk_ps[:, 0:F2], S, gw_ap[i][:],
                                 start=first, stop=last,
                                 skip_group_check=True)
                nc.tensor.matmul(blk_ps[:, F2:F2 + H2], S, wvv,
                                 start=False, stop=last,
                                 skip_group_check=True)
                gw_ap[i] = None
                if last:
                    stash2(b, blk_ps)
                    del blk_of[b]

            def stash2(b, blk_ps):
                if b % GRP == 0:
                    ng_of["num"] = ngp.tile([128, GRP, F2], BF16, tag="numbf2", name="numbf2")
                    ng_of["den"] = ngp.tile([128, GRP, H2], F32, tag="denf2", name="denf2")
                num_bf, den_f = ng_of["num"], ng_of["den"]
                nc.scalar.copy(num_bf[:, b % GRP, :], blk_ps[:, 0:F2])
                nc.vector.tensor_copy(den_f[:, b % GRP, :],
                                      blk_ps[:, F2:F2 + H2])
                if b % GRP == GRP - 1 or b == NBLK - 1:
                    g0 = (b // GRP) * GRP
                    gn = b - g0 + 1
                    rd = evp.tile([128, GRP, H2], F32, tag="rd2")
                    nc.vector.reciprocal(rd[:, 0:gn, :], den_f[:, 0:gn, :])
                    rdb = evp.tile([128, GRP, H2], BF16, tag="rdb2")
                    nc.vector.tensor_copy(rdb[:, 0:gn, :], rd[:, 0:gn, :])
                    xg = evp.tile([128, GRP, F2], BF16, tag="xg2")
                    nc.vector.tensor_tensor(
                        xg[:, 0:gn, :].rearrange("p g (h d) -> p g h d", d=cfg.D2),
                        num_bf[:, 0:gn, :].rearrange("p g (h d) -> p g h d",
                                                     d=cfg.D2),
                        rdb[:, 0:gn, :].rearrange("p g (h u) -> p g h u", u=1)
                            .to_broadcast((128, gn, H2, cfg.D2)),
                        OP.mult)
                    tm = evp.tile([128, GRP, F2], BF16, tag="tm2")
                    nc.vector.tensor_scalar(tm[:, 0:gn, :], xg[:, 0:gn, :],
                                            0.0, None, OP.min)
                    te = evp.tile([128, GRP, F2], BF16, tag="te2")
                    nc.scalar.activation(te[:, 0:gn, :], tm[:, 0:gn, :], AF.Exp)
                    nc.vector.tensor_scalar(tm[:, 0:gn, :], xg[:, 0:gn, :],
                                            0.0, -1.0, OP.max, OP.add)
                    fc = evp.tile([128, GRP, F2], BF16, tag="fc")
                    nc.vector.tensor_tensor(fc[:, 0:gn, :], te[:, 0:gn, :],
                                            tm[:, 0:gn, :], OP.add)
                    nc.vector.tensor_tensor(
                        fc[:, 0:gn, :], fc[:, 0:gn, :],
                        WFC[:].rearrange("p (u f) -> p u f", u=1)
                            .to_broadcast((128, gn, F2)),
                        OP.mult)
                    red = evp.tile([128, GRP], F32, tag="red")
                    nc.vector.tensor_reduce(
                        red[:, 0:gn].rearrange("p (g u) -> p g u", u=1),
                        fc[:, 0:gn, :], mybir.AxisListType.X, OP.add)
                    ys = evp.tile([128, GRP], F32, tag="ys")
                    nc.scalar.activation(ys[:, 0:gn], red[:, 0:gn], AF.Sigmoid,
                                         bias=BFCC[:, 0:1])
                    for j in range(gn):
                        bb = g0 + j
                        rws = min(128, cfg.SHARD - bb * 128)
                        nc.sync.dma_start(y[bb * 128:bb * 128 + rws, :],
                                          ys[0:rws, j:j + 1])

            PIPE_G = 4
            prev = None
            for i0 in range(0, NIT, PIPE_G):
                grp = list(range(i0, min(i0 + PIPE_G, NIT)))
                for i in grp:
                    kind, _b, sl, _l = items[i]
                    if kind == "edge":
                        get_window2(sl // WCH)
                if prev:
                    for i in prev:
                        emit_gw2(i)
                    for i in prev:
                        emit_scatter2(i)
                prev = grp
            for i in prev:
                emit_gw2(i)
            for i in prev:
                emit_scatter2(i)


# --------------------------------------------------------------------------
#  host entry
# --------------------------------------------------------------------------

def build(inputs, cfg: Cfg):
    ei = np.asarray(inputs["edge_index"])
    src, dst = ei[0], ei[1]
    st = prep_edges(cfg, src, dst)
    in_maps = host_inputs(cfg, st, inputs)

    nc = bacc.Bacc("TRN2", target_bir_lowering=False, debug=False,
                   num_devices=cfg.NC, dynamic_dma_scratch_size=65536)
    ins_aps = {}
    for k, v in in_maps[0].items():
        dt = mybir.dt.from_np(v.dtype)
        ins_aps[k] = nc.dram_tensor(k, list(v.shape), dt,
                                    kind="ExternalInput").ap()
    y_ap = nc.dram_tensor("y", [cfg.NBLK * 128, 1], F32,
                          kind="ExternalOutput").ap()

    with tile.TileContext(nc) as tc:
        emit_gat(tc, {"y": y_ap}, ins_aps, cfg, st)
    nc.compile()
    return nc, in_maps, st


def build_and_run(inputs, cfg: Cfg, trace=False):
    nc, in_maps, st = build(inputs, cfg)
    res = run_bass_kernel_spmd(nc, in_maps, core_ids=list(range(cfg.NC)),
                               trace=trace)
    parts = [res.results[c]["y"][:min(cfg.SHARD, cfg.N - c * cfg.SHARD)]
             for c in range(cfg.NC)]
    out = np.concatenate(parts, axis=0)
    return out, res


def kernel(**inputs):
    cfg = Cfg()
    out, _ = build_and_run(inputs, cfg)
    return out.astype(np.float32)


# revision 33
# speedup vs baseline: 1.2186x; 1.2186x over previous
"""Trainium2 Bass kernel for 2-layer GAT (nn_FAGAT) over 8 NeuronCores.

v2 design (node/dst-sharded, compact-gather message passing, bf16):
  - Core c owns dst nodes [c*SHARD, (c+1)*SHARD). Self-loop edges are handled
    densely (per dst block, S = identity); only real edges go through the
    gather pipeline.
  - Per core, the unique src set (~29k < 32768) indexes a COMPACT table, so
    one int16 index stream covers both layers:
      * L1: host ships x rows (bf16, 128 cols = 256B) at compact slots;
        per window of 16 chunks a single transpose=True dma_gather yields
        xt [feat, edge] directly (no PE transpose, no PSUM evacuation).
      * L2: after the AllGather each core re-compacts the global h2 table
        (two range-gathers, lo/hi of the sorted unique list) into a local
        compact DRAM table; edge gathers then reuse the SAME idx arrays.
  - One-hot S (bf16, DVE is_equal) turns segment softmax + weighted scatter
    into PSUM-accumulated matmuls; ST = PE-transposed S expands per-dst
    attention halves to edges.
  - Per-window batching: attention logits for 16 chunks are accumulated into
    one PSUM tile by tiny matmuls, then ONE leaky-relu + ONE exp serve the
    whole window.  PSUM evacuations ride the Activation engine.
  - Everything hot is bf16 (DVE 2x/4x modes, PE 1 cycle/row); accumulations
    stay f32 in PSUM.
"""
import os
os.environ.setdefault("NEURON_SCRATCHPAD_PAGE_SIZE", "64")
import sys
if "/opt/trn_rl_repo" not in sys.path:
    sys.path.insert(0, "/opt/trn_rl_repo")

from dataclasses import dataclass, field
import numpy as np

import concourse.bass as bass
import concourse.mybir as mybir
from concourse import bacc, tile
from concourse.bass_utils import run_bass_kernel_spmd

F32 = mybir.dt.float32
BF16 = mybir.dt.bfloat16
I16 = mybir.dt.int16
AF = mybir.ActivationFunctionType
OP = mybir.AluOpType

NEG = 0.2
EPS = 1e-16


def to_bf16(a):
    import ml_dtypes
    return np.asarray(a, dtype=np.float32).astype(ml_dtypes.bfloat16)


@dataclass
class Cfg:
    N: int = 50000
    NC: int = 8
    SPLIT: int = 32768
    KIN: int = 27
    K1: int = 32           # padded input features
    H1: int = 4
    D1: int = 64
    H2: int = 2
    D2: int = 64
    WCH: int = 16          # chunks per gather window
    GRP: int = 8           # blocks per normalization group
    CTC: int = 256         # cc table cols (bf16, 512B rows)
    timing_single_core: bool = False
    stop_after: int = 99   # debug: 1=dense, 2=L1, 3=h2, 4=gather/copy, 5=compaction

    @property
    def SHARD(self):
        return self.N // self.NC

    @property
    def NBLK(self):
        return (self.SHARD + 127) // 128

    @property
    def F1(self):
        return self.H1 * self.D1   # 256

    @property
    def F2(self):
        return self.H2 * self.D2   # 128


@dataclass
class Structure:
    nch: np.ndarray = None      # [NBLK] chunks per block (shared both layers)
    chunks: list = field(default_factory=list)  # (block, slot, first, last)
    wblocks: dict = field(default_factory=dict)  # w -> [block of each wslot]
    cores: list = field(default_factory=list)
    NCH: int = 0
    KLOC: int = 0               # compaction lo chunks (global max)
    KHIC: int = 0
    CC: int = 0                 # KLOC + KHIC


def wrap_idx(a, nch):
    """int16 idx array [nch*128] -> [128, nch*8] wrapped gather layout."""
    w = a.astype(np.int16).reshape(nch * 8, 16).T
    return np.tile(w, (8, 1)).copy()


def prep_edges(cfg: Cfg, src, dst):
    NC, SHARD, NBLK = cfg.NC, cfg.SHARD, cfg.NBLK
    src = np.asarray(src, dtype=np.int64)
    dst = np.asarray(dst, dtype=np.int64)
    per_core = []
    for c in range(NC):
        m = (dst // SHARD) == c
        es, ed = src[m], dst[m] - c * SHARD
        uniq = np.unique(es)
        assert len(uniq) < 32768, f"core {c}: {len(uniq)} unique srcs"
        cpos = np.searchsorted(uniq, es)
        k = int(np.searchsorted(uniq, cfg.SPLIT))
        blocks = []
        for b in range(NBLK):
            bm = (ed // 128) == b
            blocks.append((cpos[bm], ed[bm] - b * 128))
        per_core.append(dict(uniq=uniq, k=k, blocks=blocks))

    nch = np.zeros(NBLK, dtype=int)
    for c in range(NC):
        for b in range(NBLK):
            nch[b] = max(nch[b], -(-len(per_core[c]["blocks"][b][0]) // 128))

    st = Structure(nch=nch)
    slot = 0
    for b in range(NBLK):
        n = int(nch[b])
        for kk in range(n):
            st.chunks.append((b, slot, kk == 0, kk == n - 1))
            w, wi = divmod(slot, cfg.WCH)
            st.wblocks.setdefault(w, {})[wi] = b
            slot += 1
    st.NCH = slot

    st.KLOC = max(-(-pc["k"] // 128) for pc in per_core)
    st.KHIC = max(-(-(len(pc["uniq"]) - pc["k"]) // 128) for pc in per_core)
    st.CC = st.KLOC + st.KHIC

    NCHp = ((st.NCH + cfg.WCH - 1) // cfg.WCH) * cfg.WCH
    for c in range(NC):
        pc = per_core[c]
        uniq, k = pc["uniq"], pc["k"]
        # compact slot of unique position j
        def slot_of(j):
            return np.where(j < k, j, st.KLOC * 128 + (j - k))
        idx1 = np.zeros(NCHp * 128, np.int32)
        dl = np.full(NCHp * 128, -1.0, np.float32)
        o = 0
        for b in range(NBLK):
            cp, dloc = pc["blocks"][b]
            nb = int(nch[b])
            idx1[o:o + len(cp)] = slot_of(cp)
            dl[o:o + len(cp)] = dloc
            o += nb * 128
        # compaction gather ids: lo section then hi section (padded w/ 0)
        idxc = np.zeros(st.CC * 128, np.int32)
        idxc[0:k] = uniq[0:k]
        idxc[st.KLOC * 128:st.KLOC * 128 + (len(uniq) - k)] = uniq[k:] - cfg.SPLIT
        # compact x table (bf16 rows at compact slots)
        xrow = np.zeros((st.CC * 128, cfg.K1 * 4), np.float32)  # 128 cols
        st.cores.append(dict(
            idx1=wrap_idx(idx1, NCHp),
            dl=dl.reshape(NCHp, 128).T.copy(),     # [128, NCHp]
            idxc=wrap_idx(idxc, st.CC),
            _uniq=uniq, _k=k, _xrow=xrow,
        ))
    st.NCHp = NCHp
    return st


def host_inputs(cfg: Cfg, st: Structure, inputs):
    x = np.asarray(inputs["x"], dtype=np.float32)
    W1 = np.asarray(inputs["W1"], np.float32)
    a_src1 = np.asarray(inputs["a_src1"], np.float32)
    a_dst1 = np.asarray(inputs["a_dst1"], np.float32)
    W2 = np.asarray(inputs["W2"], np.float32)
    a_src2 = np.asarray(inputs["a_src2"], np.float32)
    a_dst2 = np.asarray(inputs["a_dst2"], np.float32)

    # W1E [K1, F1]; A1SD [K1, 8]: cols 0:4 = W1@a_dst1, 4:8 = W1@a_src1
    W1E = np.zeros((cfg.K1, cfg.F1), np.float32)
    W1E[:cfg.KIN] = W1
    A1SD = np.zeros((cfg.K1, 2 * cfg.H1), np.float32)
    for h in range(cfg.H1):
        A1SD[:cfg.KIN, h] = W1[:, h * cfg.D1:(h + 1) * cfg.D1] @ a_dst1[h]
        A1SD[:cfg.KIN, cfg.H1 + h] = W1[:, h * cfg.D1:(h + 1) * cfg.D1] @ a_src1[h]
    # W2F [F1, 132] = [W2 | W2@a_src2 | W2@a_dst2], k-tiled to [128, 2, 132]
    W2F = np.zeros((cfg.F1, cfg.F2 + 2 * cfg.H2), np.float32)
    W2F[:, :cfg.F2] = W2
    for h in range(cfg.H2):
        W2F[:, cfg.F2 + h] = W2[:, h * cfg.D2:(h + 1) * cfg.D2] @ a_src2[h]
        W2F[:, cfg.F2 + cfg.H2 + h] = W2[:, h * cfg.D2:(h + 1) * cfg.D2] @ a_dst2[h]
    W2F = np.ascontiguousarray(
        W2F.reshape(2, 128, cfg.F2 + 2 * cfg.H2).transpose(1, 0, 2))

    iota = np.tile(np.arange(128, dtype=np.float32), (128, 1))
    ident = np.eye(128, dtype=np.float32)
    wfcrow = np.tile(np.asarray(inputs["Wfc"], np.float32).reshape(1, -1), (128, 1))
    bfccol = np.full((128, 1), np.asarray(inputs["bfc"], np.float32).reshape(-1)[0],
                     dtype=np.float32)
    assert not np.any(np.asarray(inputs["b1"])) and \
        not np.any(np.asarray(inputs["b2"])), "nonzero biases unsupported"

    shared = dict(W1E=W1E, A1SD=A1SD, W2F=W2F, IOTA=iota, IDENT=ident,
                  WFCROW=wfcrow, BFCC=bfccol)

    in_maps = []
    for c in range(cfg.NC):
        m = dict(shared)
        pc = st.cores[c]
        uniq, k = pc["_uniq"], pc["_k"]
        # compact x table: rows at gapped compact slots, bf16
        xtab = np.zeros((st.CC * 128, 128), np.float32)
        xtab[0:k, :cfg.KIN] = x[uniq[0:k]]
        xtab[st.KLOC * 128:st.KLOC * 128 + len(uniq) - k, :cfg.KIN] = x[uniq[k:]]
        m["XCTAB"] = to_bf16(xtab)
        # own-shard x transposed [K1, NBLK*128]
        xtd = np.zeros((cfg.K1, cfg.NBLK * 128), np.float32)
        nrow = min(cfg.SHARD, cfg.N - c * cfg.SHARD)
        xtd[:cfg.KIN, :nrow] = x[c * cfg.SHARD:c * cfg.SHARD + nrow].T
        m["XTD"] = xtd
        m["idx1"] = pc["idx1"]
        m["idxc"] = pc["idxc"]
        # host-built one-hot S / ST, window-major layout [NW*128, WCH*128]
        dl = pc["dl"]                                   # [128, NCHp]
        NCHp = dl.shape[1]
        NW = NCHp // cfg.WCH
        S_full = (dl[:, :, None] ==
                  np.arange(128, dtype=np.float32)[None, None, :])
        Sw = S_full.reshape(128, NW, cfg.WCH, 128).transpose(1, 0, 2, 3)
        m["SH"] = to_bf16(Sw.reshape(NW * 128, cfg.WCH * 128))
        STw = S_full.transpose(2, 1, 0).reshape(128, NW, cfg.WCH, 128) \
            .transpose(1, 0, 2, 3)
        m["STH"] = to_bf16(STw.reshape(NW * 128, cfg.WCH * 128))
        in_maps.append(m)
    return in_maps


# --------------------------------------------------------------------------
#  device program
# --------------------------------------------------------------------------

def emit_gat(tc, outs, ins, cfg: Cfg, st: Structure):
    nc = tc.nc
    NBLK, F1, F2, H1, H2 = cfg.NBLK, cfg.F1, cfg.F2, cfg.H1, cfg.H2
    WCH, GRP, CTC = cfg.WCH, cfg.GRP, cfg.CTC
    NCH = st.NCH
    NW = (NCH + WCH - 1) // WCH
    y = outs["y"]

    cc_in = nc.dram_tensor("cc_in", [cfg.SHARD, CTC], BF16, kind="Internal").ap()
    cc_out = nc.dram_tensor("cc_out", [cfg.N, CTC], BF16, kind="Internal",
                            addr_space="Shared").ap()
    ctab = nc.dram_tensor("ctab", [st.CC * 128, CTC], BF16, kind="Internal").ap()

    with (
        tc.tile_pool(name="const", bufs=1) as constp,
        tc.tile_pool(name="big", bufs=1) as bigp,
    ):
        def cload(name, dtype=F32):
            src = ins[name]
            t = constp.tile(list(src.shape), dtype, tag=name)
            nc.sync.dma_start(t[:], src)
            return t

        def cload_bf(name):
            f = cload(name)
            t = constp.tile(list(ins[name].shape), BF16, tag=name + "b")
            nc.vector.tensor_copy(t[:], f[:])
            return t

        IDENT = cload_bf("IDENT")
        W1E = cload_bf("W1E")
        A1SD = cload_bf("A1SD")
        W2F = cload_bf("W2F")
        WFC = cload_bf("WFCROW")
        BFCC = cload("BFCC")
        XTD = cload_bf("XTD")
        IDX1 = cload("idx1", dtype=I16)
        IDXC = cload("idxc", dtype=I16)

        x2_all = bigp.tile([128, NBLK, F1], BF16)
        h2_sb = bigp.tile([128, NBLK, F2 + 2 * H2], BF16)
        sdss = bigp.tile([128, NBLK, 2 * H1], BF16)   # [sdst1 | ssrc1_own]
        wself = bigp.tile([128, NBLK, H1], BF16)
        w2self = bigp.tile([128, NBLK, H2], BF16)

        # ---------------- dense phase: sdst1/ssrc1_own + self weights ------
        with (
            tc.tile_pool(name="dn", bufs=1) as dnp,
            tc.tile_pool(name="ps_dn", bufs=1, space="PSUM") as psdn,
        ):
            sd_ps = psdn.tile([128, NBLK, 2 * H1], F32)
            for b in range(NBLK):
                nc.tensor.matmul(sd_ps[:, b, :], XTD[:, b * 128:(b + 1) * 128],
                                 A1SD[:], start=(b == 0), stop=(b == NBLK - 1),
                                 skip_group_check=True)
            nc.scalar.copy(sdss[:], sd_ps[:])
            tself = dnp.tile([128, NBLK, H1], BF16)
            nc.vector.tensor_tensor(tself[:], sdss[:, :, 0:H1],
                                    sdss[:, :, H1:2 * H1], OP.add)
            lr = dnp.tile([128, NBLK, H1], BF16)
            nc.vector.scalar_tensor_tensor(lr[:], tself[:], NEG, tself[:],
                                           OP.mult, OP.max)
            nc.scalar.activation(wself[:], lr[:], AF.Exp)
        if cfg.stop_after <= 1:
            nc.gpsimd.dma_start(y[0:128, :], wself[0:128, 0, 0:1])
            return

        # ---------------- layer 1 ----------------
        with (
            tc.tile_pool(name="l1g", bufs=3) as gpool,
            tc.tile_pool(name="l1s", bufs=3) as spool,
            tc.tile_pool(name="l1w", bufs=3) as wpool,
            tc.tile_pool(name="l1gw", bufs=10) as gwpool,
            tc.tile_pool(name="l1ng", bufs=2) as ngp,
            tc.tile_pool(name="l1ev", bufs=1) as evp,
            tc.tile_pool(name="ps_hs", bufs=4, space="PSUM") as pshs,
            tc.tile_pool(name="ps_ss", bufs=2, space="PSUM") as psss,
            tc.tile_pool(name="ps_blk", bufs=2, space="PSUM") as psblk,
        ):
            windows = {}

            def get_window(w):
                if w in windows:
                    return windows[w]
                n = min(WCH, NCH - w * WCH)
                xtw = gpool.tile([128, 1, WCH * 128], BF16, tag="xtw")
                # transposed gathers crash above 512 idxs -> 4-chunk pieces
                for q0 in range(0, n, 4):
                    qn = min(4, n - q0)
                    nc.gpsimd.dma_gather(
                        xtw[:, :, q0 * 128:(q0 + qn) * 128], ins["XCTAB"],
                        IDX1[:, (w * WCH + q0) * 8:(w * WCH + q0 + qn) * 8],
                        qn * 128, qn * 128, 128, transpose=True)
                Sb = spool.tile([128, WCH, 128], BF16, tag="Sb")
                nc.sync.dma_start(
                    Sb[:].rearrange("p c j -> p (c j)")[:, 0:n * 128],
                    ins["SH"][w * 128:(w + 1) * 128, 0:n * 128])
                STb = spool.tile([128, WCH, 128], BF16, tag="STb")
                nc.sync.dma_start(
                    STb[:].rearrange("p c j -> p (c j)")[:, 0:n * 128],
                    ins["STH"][w * 128:(w + 1) * 128, 0:n * 128])
                # window logits: ssrc (tiny matmul) + sdst expand, batched
                ss_ps = psss.tile([128, WCH, H1], F32, tag="ssps")
                for ci in range(n):
                    bb = st.wblocks[w][ci]
                    nc.tensor.matmul(ss_ps[:, ci, :],
                                     xtw[0:cfg.K1, 0, ci * 128:(ci + 1) * 128],
                                     A1SD[:, H1:2 * H1],
                                     start=(ci == 0), stop=False,
                                     skip_group_check=True)
                    nc.tensor.matmul(ss_ps[:, ci, :], STb[:, ci, :],
                                     sdss[:, bb, 0:H1],
                                     start=False, stop=(ci == n - 1),
                                     skip_group_check=True)
                ss_sb = wpool.tile([128, WCH, H1], F32, tag="sssb")
                nc.scalar.copy(ss_sb[:, 0:n, :], ss_ps[:, 0:n, :])
                lr = wpool.tile([128, WCH, H1], F32, tag="lr")
                nc.vector.scalar_tensor_tensor(lr[:, 0:n, :], ss_sb[:, 0:n, :],
                                               NEG, ss_sb[:, 0:n, :],
                                               OP.mult, OP.max)
                wv = wpool.tile([128, WCH, H1], BF16, tag="wv")
                nc.scalar.activation(wv[:, 0:n, :], lr[:, 0:n, :], AF.Exp)
                windows[w] = (xtw, Sb, wv)
                return windows[w]

            # item stream: per block a dense self pseudo-chunk then edge chunks
            items = []
            slot = 0
            for b in range(NBLK):
                nb = int(st.nch[b])
                items.append(("self", b, None, nb == 0))
                for kk in range(nb):
                    items.append(("edge", b, slot, kk == nb - 1))
                    slot += 1
            NIT = len(items)
            NWA = (NCH + WCH - 1) // WCH
            ng_of = {}

            def stash(b, blk_ps):
                if b % GRP == 0:
                    ng_of["num"] = ngp.tile([128, GRP, F1], BF16, tag="numbf",
                                            name="numbf")
                    ng_of["den"] = ngp.tile([128, GRP, H1], F32, tag="denf",
                                            name="denf")
                num_bf, den_f = ng_of["num"], ng_of["den"]
                nc.scalar.copy(num_bf[:, b % GRP, :], blk_ps[:, 0:F1])
                nc.vector.tensor_copy(den_f[:, b % GRP, :],
                                      blk_ps[:, F1:F1 + H1])
                if b % GRP == GRP - 1 or b == NBLK - 1:
                    g0 = (b // GRP) * GRP
                    gn = b - g0 + 1
                    rd = evp.tile([128, GRP, H1], F32, tag="rd")
                    nc.vector.reciprocal(rd[:, 0:gn, :], den_f[:, 0:gn, :])
                    rdb = evp.tile([128, GRP, H1], BF16, tag="rdb")
                    nc.vector.tensor_copy(rdb[:, 0:gn, :], rd[:, 0:gn, :])
                    xg = evp.tile([128, GRP, F1], BF16, tag="xg")
                    nc.vector.tensor_tensor(
                        xg[:, 0:gn, :].rearrange("p g (h d) -> p g h d", d=cfg.D1),
                        num_bf[:, 0:gn, :].rearrange("p g (h d) -> p g h d",
                                                     d=cfg.D1),
                        rdb[:, 0:gn, :].rearrange("p g (h u) -> p g h u", u=1)
                            .to_broadcast((128, gn, H1, cfg.D1)),
                        OP.mult)
                    # elu: exp(min(x,0)) - 1 + max(x,0)
                    tm = evp.tile([128, GRP, F1], BF16, tag="tm")
                    nc.vector.tensor_scalar(tm[:, 0:gn, :], xg[:, 0:gn, :],
                                            0.0, None, OP.min)
                    te = evp.tile([128, GRP, F1], BF16, tag="te")
                    nc.scalar.activation(te[:, 0:gn, :], tm[:, 0:gn, :], AF.Exp)
                    nc.vector.tensor_scalar(tm[:, 0:gn, :], xg[:, 0:gn, :],
                                            0.0, -1.0, OP.max, OP.add)
                    nc.vector.tensor_tensor(x2_all[:, g0:g0 + gn, :],
                                            te[:, 0:gn, :], tm[:, 0:gn, :],
                                            OP.add)

            blk_ps = None
            for i in range(NIT):
                kind, b, sl, last = items[i]
                if kind == "edge":
                    w, wi = divmod(sl, WCH)
                    xtw, Sb, wv = get_window(w)
                    if w + 1 < NWA:
                        get_window(w + 1)     # prefetch next window
                    lhsT = xtw[0:cfg.K1, 0, wi * 128:(wi + 1) * 128]
                    S = Sb[:, wi, :]
                    wvv = wv[:, wi, :]
                else:
                    lhsT = XTD[:, b * 128:(b + 1) * 128]
                    S = IDENT[:]
                    wvv = wself[:, b, :]
                hs_ps = pshs.tile([128, F1], F32, tag="hs")
                nc.tensor.matmul(hs_ps[:], lhsT, W1E[:], start=True, stop=True,
                                 skip_group_check=True)
                gw = gwpool.tile([128, F1], BF16, tag="gw")
                if i % 2 == 0:
                    src_ap = hs_ps[:]
                else:
                    hs_bf = gwpool.tile([128, F1], BF16, tag="hsbf")
                    nc.scalar.copy(hs_bf[:], hs_ps[:])
                    src_ap = hs_bf[:]
                nc.vector.tensor_tensor(
                    gw[:].rearrange("p (h d) -> p h d", d=cfg.D1),
                    src_ap.rearrange("p (h d) -> p h d", d=cfg.D1),
                    wvv.rearrange("p (h u) -> p h u", u=1)
                        .to_broadcast((128, H1, cfg.D1)),
                    OP.mult)
                if kind == "self":
                    blk_ps = psblk.tile([128, F1 + H1], F32, tag="blk")
                nc.tensor.matmul(blk_ps[:, 0:F1], S, gw[:],
                                 start=(kind == "self"), stop=last,
                                 skip_group_check=True)
                nc.tensor.matmul(blk_ps[:, F1:F1 + H1], S, wvv,
                                 start=False, stop=last,
                                 skip_group_check=True)
                if last:
                    stash(b, blk_ps)

        if cfg.stop_after <= 2:
            nc.gpsimd.dma_start(y[0:128, :], x2_all[0:128, 0, 0:1])
            return

        # ---------------- h2 build + AllGather + compaction ----------------
        F2E = F2 + 2 * H2
        with (
            tc.tile_pool(name="h2sb", bufs=2) as hsb,
            tc.tile_pool(name="ps_h2", bufs=2, space="PSUM") as psh,
            tc.tile_pool(name="ps_h2t", bufs=2, space="PSUM") as psht,
        ):
            for b in range(NBLK):
                rows = min(128, cfg.SHARD - b * 128)
                xt2_ps = psht.tile([128, 2, 128], BF16, tag="x2t")
                for k in range(2):
                    nc.tensor.transpose(xt2_ps[:, k, :],
                                        x2_all[:, b, k * 128:(k + 1) * 128],
                                        IDENT[:])
                xt2 = hsb.tile([128, 2, 128], BF16, tag="x2sb")
                nc.vector.tensor_copy(xt2[:], xt2_ps[:])
                h2_ps = psh.tile([128, F2E], F32, tag="h2ps")
                for k in range(2):
                    nc.tensor.matmul(h2_ps[:], xt2[:, k, :], W2F[:, k, :],
                                     start=(k == 0), stop=(k == 1),
                                     skip_group_check=True)
                nc.scalar.copy(h2_sb[:, b, :], h2_ps[:])
                nc.sync.dma_start(cc_in[b * 128:b * 128 + rows, 0:F2 + H2],
                                  h2_sb[0:rows, b, 0:F2 + H2])
            # self weights for layer 2
            t2 = hsb.tile([128, NBLK, H2], BF16, tag="t2")
            nc.vector.tensor_tensor(t2[:], h2_sb[:, :, F2:F2 + H2],
                                    h2_sb[:, :, F2 + H2:F2E], OP.add)
            lr2 = hsb.tile([128, NBLK, H2], BF16, tag="lr2")
            nc.vector.scalar_tensor_tensor(lr2[:], t2[:], NEG, t2[:],
                                           OP.mult, OP.max)
            nc.scalar.activation(w2self[:], lr2[:], AF.Exp)

        if cfg.stop_after <= 3:
            nc.gpsimd.dma_start(y[0:128, :], h2_sb[0:128, 0, 0:1])
            return

        if cfg.timing_single_core:
            nc.sync.dma_start(cc_out[0:cfg.SHARD, :], cc_in[:])
        else:
            nc.gpsimd.collective_compute(
                "AllGather", OP.bypass,
                replica_groups=[list(range(cfg.NC))],
                ins=[cc_in[:]],
                outs=[cc_out[:]],
            )

        if cfg.stop_after <= 4:
            nc.gpsimd.dma_start(y[0:128, :], h2_sb[0:128, 0, 0:1])
            return

        # compaction: gather unique rows from cc_out into local compact ctab
        with tc.tile_pool(name="cg", bufs=4) as cgp:
            ctv = ctab.rearrange("(c p) f -> p c f", p=128)
            F2U = F2 + H2    # used cols of a cc row
            for sec, base, ncc in ((0, 0, st.KLOC), (1, st.KLOC, st.KHIC)):
                tab = cc_out[0:cfg.SPLIT, :] if sec == 0 else \
                    cc_out[cfg.SPLIT:cfg.N, :]
                for w0 in range(0, ncc, WCH):
                    n = min(WCH, ncc - w0)
                    gt = cgp.tile([128, WCH, CTC], BF16, tag="cgt")
                    for q0 in range(0, n, 8):  # <=1024 idxs per gather
                        qn = min(8, n - q0)
                        nc.gpsimd.dma_gather(
                            gt[:, q0:q0 + qn, :], tab,
                            IDXC[:, (base + w0 + q0) * 8:
                                 (base + w0 + q0 + qn) * 8],
                            qn * 128, qn * 128, CTC)
                    nc.sync.dma_start(
                        ctv[:, base + w0:base + w0 + n, 0:F2U],
                        gt[:, 0:n, 0:F2U])

        if cfg.stop_after <= 5:
            nc.gpsimd.dma_start(y[0:128, :], h2_sb[0:128, 0, 0:1])
            return

        # ---------------- layer 2 ----------------
        with (
            tc.tile_pool(name="l2g", bufs=3) as gpool,
            tc.tile_pool(name="l2s", bufs=3) as spool,
            tc.tile_pool(name="l2w", bufs=3) as wpool,
            tc.tile_pool(name="l2gw", bufs=10) as gwpool,
            tc.tile_pool(name="l2ng", bufs=2) as ngp,
            tc.tile_pool(name="l2ev", bufs=1) as evp,
            tc.tile_pool(name="ps_ss2", bufs=3, space="PSUM") as psss,
            tc.tile_pool(name="ps_blk2", bufs=2, space="PSUM") as psblk,
        ):
            windows = {}

            def get_window2(w):
                if w in windows:
                    return windows[w]
                n = min(WCH, NCH - w * WCH)
                gt = gpool.tile([128, WCH, CTC], BF16, tag="gt")
                for q0 in range(0, n, 8):  # <=1024 idxs per gather
                    qn = min(8, n - q0)
                    nc.gpsimd.dma_gather(
                        gt[:, q0:q0 + qn, :], ctab,
                        IDX1[:, (w * WCH + q0) * 8:(w * WCH + q0 + qn) * 8],
                        qn * 128, qn * 128, CTC)
                Sb = spool.tile([128, WCH, 128], BF16, tag="Sb2")
                nc.sync.dma_start(
                    Sb[:].rearrange("p c j -> p (c j)")[:, 0:n * 128],
                    ins["SH"][w * 128:(w + 1) * 128, 0:n * 128])
                STb = spool.tile([128, WCH, 128], BF16, tag="STb2")
                nc.sync.dma_start(
                    STb[:].rearrange("p c j -> p (c j)")[:, 0:n * 128],
                    ins["STH"][w * 128:(w + 1) * 128, 0:n * 128])
                ss_ps = psss.tile([128, WCH, H2], F32, tag="ssps2")
                for ci in range(n):
                    bb = st.wblocks[w][ci]
                    nc.tensor.matmul(ss_ps[:, ci, :], STb[:, ci, :],
                                     h2_sb[:, bb, F2 + H2:F2E],
                                     start=(ci == 0), stop=(ci == n - 1),
                                     skip_group_check=True)
                t = wpool.tile([128, WCH, H2], F32, tag="t")
                nc.vector.tensor_tensor(t[:, 0:n, :], ss_ps[:, 0:n, :],
                                        gt[:, 0:n, F2:F2 + H2], OP.add)
                lr = wpool.tile([128, WCH, H2], F32, tag="lr")
                nc.vector.scalar_tensor_tensor(lr[:, 0:n, :], t[:, 0:n, :],
                                               NEG, t[:, 0:n, :],
                                               OP.mult, OP.max)
                wv = wpool.tile([128, WCH, H2], BF16, tag="wv")
                nc.scalar.activation(wv[:, 0:n, :], lr[:, 0:n, :], AF.Exp)
                windows[w] = (gt, Sb, wv)
                return windows[w]

            items = []
            slot = 0
            for b in range(NBLK):
                nb = int(st.nch[b])
                items.append(("self", b, None, nb == 0))
                for kk in range(nb):
                    items.append(("edge", b, slot, kk == nb - 1))
                    slot += 1
            NIT = len(items)
            NWA = (NCH + WCH - 1) // WCH
            ng_of = {}

            def stash2(b, blk_ps):
                if b % GRP == 0:
                    ng_of["num"] = ngp.tile([128, GRP, F2], BF16, tag="numbf2",
                                            name="numbf2")
                    ng_of["den"] = ngp.tile([128, GRP, H2], F32, tag="denf2",
                                            name="denf2")
                num_bf, den_f = ng_of["num"], ng_of["den"]
                nc.scalar.copy(num_bf[:, b % GRP, :], blk_ps[:, 0:F2])
                nc.vector.tensor_copy(den_f[:, b % GRP, :],
                                      blk_ps[:, F2:F2 + H2])
                if b % GRP == GRP - 1 or b == NBLK - 1:
                    g0 = (b // GRP) * GRP
                    gn = b - g0 + 1
                    rd = evp.tile([128, GRP, H2], F32, tag="rd2")
                    nc.vector.reciprocal(rd[:, 0:gn, :], den_f[:, 0:gn, :])
                    rdb = evp.tile([128, GRP, H2], BF16, tag="rdb2")
                    nc.vector.tensor_copy(rdb[:, 0:gn, :], rd[:, 0:gn, :])
                    xg = evp.tile([128, GRP, F2], BF16, tag="xg2")
                    nc.vector.tensor_tensor(
                        xg[:, 0:gn, :].rearrange("p g (h d) -> p g h d", d=cfg.D2),
                        num_bf[:, 0:gn, :].rearrange("p g (h d) -> p g h d",
                                                     d=cfg.D2),
                        rdb[:, 0:gn, :].rearrange("p g (h u) -> p g h u", u=1)
                            .to_broadcast((128, gn, H2, cfg.D2)),
                        OP.mult)
                    tm = evp.tile([128, GRP, F2], BF16, tag="tm2")
                    nc.vector.tensor_scalar(tm[:, 0:gn, :], xg[:, 0:gn, :],
                                            0.0, None, OP.min)
                    te = evp.tile([128, GRP, F2], BF16, tag="te2")
                    nc.scalar.activation(te[:, 0:gn, :], tm[:, 0:gn, :], AF.Exp)
                    nc.vector.tensor_scalar(tm[:, 0:gn, :], xg[:, 0:gn, :],
                                            0.0, -1.0, OP.max, OP.add)
                    fc = evp.tile([128, GRP, F2], BF16, tag="fc")
                    nc.vector.tensor_tensor(fc[:, 0:gn, :], te[:, 0:gn, :],
                                            tm[:, 0:gn, :], OP.add)
                    nc.vector.tensor_tensor(
                        fc[:, 0:gn, :], fc[:, 0:gn, :],
                        WFC[:].rearrange("p (u f) -> p u f", u=1)
                            .to_broadcast((128, gn, F2)),
                        OP.mult)
                    red = evp.tile([128, GRP], F32, tag="red")
                    nc.vector.tensor_reduce(
                        red[:, 0:gn].rearrange("p (g u) -> p g u", u=1),
                        fc[:, 0:gn, :], mybir.AxisListType.X, OP.add)
                    ys = evp.tile([128, GRP], F32, tag="ys")
                    nc.scalar.activation(ys[:, 0:gn], red[:, 0:gn], AF.Sigmoid,
                                         bias=BFCC[:, 0:1])
                    for j in range(gn):
                        bb = g0 + j
                        rws = min(128, cfg.SHARD - bb * 128)
                        nc.sync.dma_start(y[bb * 128:bb * 128 + rws, :],
                                          ys[0:rws, j:j + 1])

            blk_ps = None
            for i in range(NIT):
                kind, b, sl, last = items[i]
                if kind == "edge":
                    w, wi = divmod(sl, WCH)
                    gt, Sb, wv = get_window2(w)
                    if w + 1 < NWA:
                        get_window2(w + 1)    # prefetch next window
                    src_ap = gt[:, wi, 0:F2]
                    S = Sb[:, wi, :]
                    wvv = wv[:, wi, :]
                else:
                    src_ap = h2_sb[:, b, 0:F2]
                    S = IDENT[:]
                    wvv = w2self[:, b, :]
                gw = gwpool.tile([128, F2], BF16, tag="gw2")
                nc.vector.tensor_tensor(
                    gw[:].rearrange("p (h d) -> p h d", d=cfg.D2),
                    src_ap.rearrange("p (h d) -> p h d", d=cfg.D2),
                    wvv.rearrange("p (h u) -> p h u", u=1)
                        .to_broadcast((128, H2, cfg.D2)),
                    OP.mult)
                if kind == "self":
                    blk_ps = psblk.tile([128, F2 + H2], F32, tag="blk2")
                nc.tensor.matmul(blk_ps[:, 0:F2], S, gw[:],
                                 start=(kind == "self"), stop=last,
                                 skip_group_check=True)
                nc.tensor.matmul(blk_ps[:, F2:F2 + H2], S, wvv,
                                 start=False, stop=last,
                                 skip_group_check=True)
                if last:
                    stash2(b, blk_ps)


# --------------------------------------------------------------------------
#  host entry
# --------------------------------------------------------------------------

def build(inputs, cfg: Cfg):
    ei = np.asarray(inputs["edge_index"])
    src, dst = ei[0], ei[1]
    st = prep_edges(cfg, src, dst)
    in_maps = host_inputs(cfg, st, inputs)

    nc = bacc.Bacc("TRN2", target_bir_lowering=False, debug=False,
                   num_devices=cfg.NC, dynamic_dma_scratch_size=65536)
    ins_aps = {}
    for k, v in in_maps[0].items():
        dt = mybir.dt.from_np(v.dtype)
        ins_aps[k] = nc.dram_tensor(k, list(v.shape), dt,
                                    kind="ExternalInput").ap()
    y_ap = nc.dram_tensor("y", [cfg.NBLK * 128, 1], F32,
                          kind="ExternalOutput").ap()

    with tile.TileContext(nc) as tc:
        emit_gat(tc, {"y": y_ap}, ins_aps, cfg, st)
    nc.compile()
    return nc, in_maps, st


def build_and_run(inputs, cfg: Cfg, trace=False):
    nc, in_maps, st = build(inputs, cfg)
    res = run_bass_kernel_spmd(nc, in_maps, core_ids=list(range(cfg.NC)),
                               trace=trace)
    parts = [res.results[c]["y"][:min(cfg.SHARD, cfg.N - c * cfg.SHARD)]
             for c in range(cfg.NC)]
    out = np.concatenate(parts, axis=0)
    return out, res


def kernel(**inputs):
    cfg = Cfg()
    out, _ = build_and_run(inputs, cfg)
    return out.astype(np.float32)


# revision 34
# speedup vs baseline: 1.2192x; 1.0005x over previous
"""Trainium2 Bass kernel for 2-layer GAT (nn_FAGAT) over 8 NeuronCores.

v2 design (node/dst-sharded, compact-gather message passing, bf16):
  - Core c owns dst nodes [c*SHARD, (c+1)*SHARD). Self-loop edges are handled
    densely (per dst block, S = identity); only real edges go through the
    gather pipeline.
  - Per core, the unique src set (~29k < 32768) indexes a COMPACT table, so
    one int16 index stream covers both layers:
      * L1: host ships x rows (bf16, 128 cols = 256B) at compact slots;
        per window of 16 chunks a single transpose=True dma_gather yields
        xt [feat, edge] directly (no PE transpose, no PSUM evacuation).
      * L2: after the AllGather each core re-compacts the global h2 table
        (two range-gathers, lo/hi of the sorted unique list) into a local
        compact DRAM table; edge gathers then reuse the SAME idx arrays.
  - One-hot S (bf16, DVE is_equal) turns segment softmax + weighted scatter
    into PSUM-accumulated matmuls; ST = PE-transposed S expands per-dst
    attention halves to edges.
  - Per-window batching: attention logits for 16 chunks are accumulated into
    one PSUM tile by tiny matmuls, then ONE leaky-relu + ONE exp serve the
    whole window.  PSUM evacuations ride the Activation engine.
  - Everything hot is bf16 (DVE 2x/4x modes, PE 1 cycle/row); accumulations
    stay f32 in PSUM.
"""
import os
os.environ.setdefault("NEURON_SCRATCHPAD_PAGE_SIZE", "64")
import sys
if "/opt/trn_rl_repo" not in sys.path:
    sys.path.insert(0, "/opt/trn_rl_repo")

from dataclasses import dataclass, field
import numpy as np

import concourse.bass as bass
import concourse.mybir as mybir
from concourse import bacc, tile
from concourse.bass_utils import run_bass_kernel_spmd

F32 = mybir.dt.float32
BF16 = mybir.dt.bfloat16
I16 = mybir.dt.int16
AF = mybir.ActivationFunctionType
OP = mybir.AluOpType

NEG = 0.2
EPS = 1e-16


def to_bf16(a):
    import ml_dtypes
    return np.asarray(a, dtype=np.float32).astype(ml_dtypes.bfloat16)


@dataclass
class Cfg:
    N: int = 50000
    NC: int = 8
    SPLIT: int = 32768
    KIN: int = 27
    K1: int = 32           # padded input features
    H1: int = 4
    D1: int = 64
    H2: int = 2
    D2: int = 64
    WCH: int = 16          # chunks per gather window
    GRP: int = 8           # blocks per normalization group
    CTC: int = 256         # cc table cols (bf16, 512B rows)
    timing_single_core: bool = False
    stop_after: int = 99   # debug: 1=dense, 2=L1, 3=h2, 4=gather/copy, 5=compaction

    @property
    def SHARD(self):
        return self.N // self.NC

    @property
    def NBLK(self):
        return (self.SHARD + 127) // 128

    @property
    def F1(self):
        return self.H1 * self.D1   # 256

    @property
    def F2(self):
        return self.H2 * self.D2   # 128


@dataclass
class Structure:
    nch: np.ndarray = None      # [NBLK] chunks per block (shared both layers)
    chunks: list = field(default_factory=list)  # (block, slot, first, last)
    wblocks: dict = field(default_factory=dict)  # w -> [block of each wslot]
    cores: list = field(default_factory=list)
    NCH: int = 0
    KLOC: int = 0               # compaction lo chunks (global max)
    KHIC: int = 0
    CC: int = 0                 # KLOC + KHIC


def wrap_idx(a, nch):
    """int16 idx array [nch*128] -> [128, nch*8] wrapped gather layout."""
    w = a.astype(np.int16).reshape(nch * 8, 16).T
    return np.tile(w, (8, 1)).copy()


def prep_edges(cfg: Cfg, src, dst):
    NC, SHARD, NBLK = cfg.NC, cfg.SHARD, cfg.NBLK
    src = np.asarray(src, dtype=np.int64)
    dst = np.asarray(dst, dtype=np.int64)
    per_core = []
    for c in range(NC):
        m = (dst // SHARD) == c
        es, ed = src[m], dst[m] - c * SHARD
        uniq = np.unique(es)
        assert len(uniq) < 32768, f"core {c}: {len(uniq)} unique srcs"
        cpos = np.searchsorted(uniq, es)
        k = int(np.searchsorted(uniq, cfg.SPLIT))
        blocks = []
        for b in range(NBLK):
            bm = (ed // 128) == b
            blocks.append((cpos[bm], ed[bm] - b * 128))
        per_core.append(dict(uniq=uniq, k=k, blocks=blocks))

    nch = np.zeros(NBLK, dtype=int)
    for c in range(NC):
        for b in range(NBLK):
            nch[b] = max(nch[b], -(-len(per_core[c]["blocks"][b][0]) // 128))

    st = Structure(nch=nch)
    slot = 0
    for b in range(NBLK):
        n = int(nch[b])
        for kk in range(n):
            st.chunks.append((b, slot, kk == 0, kk == n - 1))
            w, wi = divmod(slot, cfg.WCH)
            st.wblocks.setdefault(w, {})[wi] = b
            slot += 1
    st.NCH = slot

    st.KLOC = max(-(-pc["k"] // 128) for pc in per_core)
    st.KHIC = max(-(-(len(pc["uniq"]) - pc["k"]) // 128) for pc in per_core)
    st.CC = st.KLOC + st.KHIC

    NCHp = ((st.NCH + cfg.WCH - 1) // cfg.WCH) * cfg.WCH
    for c in range(NC):
        pc = per_core[c]
        uniq, k = pc["uniq"], pc["k"]
        # compact slot of unique position j
        def slot_of(j):
            return np.where(j < k, j, st.KLOC * 128 + (j - k))
        idx1 = np.zeros(NCHp * 128, np.int32)
        dl = np.full(NCHp * 128, -1.0, np.float32)
        o = 0
        for b in range(NBLK):
            cp, dloc = pc["blocks"][b]
            nb = int(nch[b])
            idx1[o:o + len(cp)] = slot_of(cp)
            dl[o:o + len(cp)] = dloc
            o += nb * 128
        # compaction gather ids: lo section then hi section (padded w/ 0)
        idxc = np.zeros(st.CC * 128, np.int32)
        idxc[0:k] = uniq[0:k]
        idxc[st.KLOC * 128:st.KLOC * 128 + (len(uniq) - k)] = uniq[k:] - cfg.SPLIT
        # compact x table (bf16 rows at compact slots)
        xrow = np.zeros((st.CC * 128, cfg.K1 * 4), np.float32)  # 128 cols
        st.cores.append(dict(
            idx1=wrap_idx(idx1, NCHp),
            dl=dl.reshape(NCHp, 128).T.copy(),     # [128, NCHp]
            idxc=wrap_idx(idxc, st.CC),
            _uniq=uniq, _k=k, _xrow=xrow,
        ))
    st.NCHp = NCHp
    return st


def host_inputs(cfg: Cfg, st: Structure, inputs):
    x = np.asarray(inputs["x"], dtype=np.float32)
    W1 = np.asarray(inputs["W1"], np.float32)
    a_src1 = np.asarray(inputs["a_src1"], np.float32)
    a_dst1 = np.asarray(inputs["a_dst1"], np.float32)
    W2 = np.asarray(inputs["W2"], np.float32)
    a_src2 = np.asarray(inputs["a_src2"], np.float32)
    a_dst2 = np.asarray(inputs["a_dst2"], np.float32)

    # W1E [K1, F1]; A1SD [K1, 8]: cols 0:4 = W1@a_dst1, 4:8 = W1@a_src1
    W1E = np.zeros((cfg.K1, cfg.F1), np.float32)
    W1E[:cfg.KIN] = W1
    A1SD = np.zeros((cfg.K1, 2 * cfg.H1), np.float32)
    for h in range(cfg.H1):
        A1SD[:cfg.KIN, h] = W1[:, h * cfg.D1:(h + 1) * cfg.D1] @ a_dst1[h]
        A1SD[:cfg.KIN, cfg.H1 + h] = W1[:, h * cfg.D1:(h + 1) * cfg.D1] @ a_src1[h]
    # W2F [F1, 132] = [W2 | W2@a_src2 | W2@a_dst2], k-tiled to [128, 2, 132]
    W2F = np.zeros((cfg.F1, cfg.F2 + 2 * cfg.H2), np.float32)
    W2F[:, :cfg.F2] = W2
    for h in range(cfg.H2):
        W2F[:, cfg.F2 + h] = W2[:, h * cfg.D2:(h + 1) * cfg.D2] @ a_src2[h]
        W2F[:, cfg.F2 + cfg.H2 + h] = W2[:, h * cfg.D2:(h + 1) * cfg.D2] @ a_dst2[h]
    W2F = np.ascontiguousarray(
        W2F.reshape(2, 128, cfg.F2 + 2 * cfg.H2).transpose(1, 0, 2))

    iota = np.tile(np.arange(128, dtype=np.float32), (128, 1))
    ident = np.eye(128, dtype=np.float32)
    wfcrow = np.tile(np.asarray(inputs["Wfc"], np.float32).reshape(1, -1), (128, 1))
    bfccol = np.full((128, 1), np.asarray(inputs["bfc"], np.float32).reshape(-1)[0],
                     dtype=np.float32)
    assert not np.any(np.asarray(inputs["b1"])) and \
        not np.any(np.asarray(inputs["b2"])), "nonzero biases unsupported"

    shared = dict(W1E=W1E, A1SD=A1SD, W2F=W2F, IOTA=iota, IDENT=ident,
                  WFCROW=wfcrow, BFCC=bfccol)

    in_maps = []
    for c in range(cfg.NC):
        m = dict(shared)
        pc = st.cores[c]
        uniq, k = pc["_uniq"], pc["_k"]
        # compact x table: rows at gapped compact slots, bf16
        xtab = np.zeros((st.CC * 128, 128), np.float32)
        xtab[0:k, :cfg.KIN] = x[uniq[0:k]]
        xtab[st.KLOC * 128:st.KLOC * 128 + len(uniq) - k, :cfg.KIN] = x[uniq[k:]]
        m["XCTAB"] = to_bf16(xtab)
        # own-shard x transposed [K1, NBLK*128]
        xtd = np.zeros((cfg.K1, cfg.NBLK * 128), np.float32)
        nrow = min(cfg.SHARD, cfg.N - c * cfg.SHARD)
        xtd[:cfg.KIN, :nrow] = x[c * cfg.SHARD:c * cfg.SHARD + nrow].T
        m["XTD"] = xtd
        m["idx1"] = pc["idx1"]
        m["idxc"] = pc["idxc"]
        # host-built one-hot S / ST, window-major layout [NW*128, WCH*128]
        dl = pc["dl"]                                   # [128, NCHp]
        NCHp = dl.shape[1]
        NW = NCHp // cfg.WCH
        S_full = (dl[:, :, None] ==
                  np.arange(128, dtype=np.float32)[None, None, :])
        Sw = S_full.reshape(128, NW, cfg.WCH, 128).transpose(1, 0, 2, 3)
        m["SH"] = to_bf16(Sw.reshape(NW * 128, cfg.WCH * 128))
        STw = S_full.transpose(2, 1, 0).reshape(128, NW, cfg.WCH, 128) \
            .transpose(1, 0, 2, 3)
        m["STH"] = to_bf16(STw.reshape(NW * 128, cfg.WCH * 128))
        in_maps.append(m)
    return in_maps


# --------------------------------------------------------------------------
#  device program
# --------------------------------------------------------------------------

def emit_gat(tc, outs, ins, cfg: Cfg, st: Structure):
    nc = tc.nc
    NBLK, F1, F2, H1, H2 = cfg.NBLK, cfg.F1, cfg.F2, cfg.H1, cfg.H2
    WCH, GRP, CTC = cfg.WCH, cfg.GRP, cfg.CTC
    NCH = st.NCH
    NW = (NCH + WCH - 1) // WCH
    y = outs["y"]

    cc_in = nc.dram_tensor("cc_in", [cfg.SHARD, CTC], BF16, kind="Internal").ap()
    cc_out = nc.dram_tensor("cc_out", [cfg.N, CTC], BF16, kind="Internal",
                            addr_space="Shared").ap()
    ctab = nc.dram_tensor("ctab", [st.CC * 128, CTC], BF16, kind="Internal").ap()

    with (
        tc.tile_pool(name="const", bufs=1) as constp,
        tc.tile_pool(name="big", bufs=1) as bigp,
    ):
        def cload(name, dtype=F32):
            src = ins[name]
            t = constp.tile(list(src.shape), dtype, tag=name)
            nc.sync.dma_start(t[:], src)
            return t

        def cload_bf(name):
            f = cload(name)
            t = constp.tile(list(ins[name].shape), BF16, tag=name + "b")
            nc.vector.tensor_copy(t[:], f[:])
            return t

        IDENT = cload_bf("IDENT")
        W1E = cload_bf("W1E")
        A1SD = cload_bf("A1SD")
        W2F = cload_bf("W2F")
        WFC = cload_bf("WFCROW")
        BFCC = cload("BFCC")
        XTD = cload_bf("XTD")
        IDX1 = cload("idx1", dtype=I16)
        IDXC = cload("idxc", dtype=I16)

        x2_all = bigp.tile([128, NBLK, F1], BF16)
        h2_sb = bigp.tile([128, NBLK, F2 + 2 * H2], BF16)
        sdss = bigp.tile([128, NBLK, 2 * H1], BF16)   # [sdst1 | ssrc1_own]
        wself = bigp.tile([128, NBLK, H1], BF16)
        w2self = bigp.tile([128, NBLK, H2], BF16)

        # ---------------- dense phase: sdst1/ssrc1_own + self weights ------
        with (
            tc.tile_pool(name="dn", bufs=1) as dnp,
            tc.tile_pool(name="ps_dn", bufs=1, space="PSUM") as psdn,
        ):
            sd_ps = psdn.tile([128, NBLK, 2 * H1], F32)
            for b in range(NBLK):
                nc.tensor.matmul(sd_ps[:, b, :], XTD[:, b * 128:(b + 1) * 128],
                                 A1SD[:], start=(b == 0), stop=(b == NBLK - 1),
                                 skip_group_check=True)
            nc.scalar.copy(sdss[:], sd_ps[:])
            tself = dnp.tile([128, NBLK, H1], BF16)
            nc.vector.tensor_tensor(tself[:], sdss[:, :, 0:H1],
                                    sdss[:, :, H1:2 * H1], OP.add)
            lr = dnp.tile([128, NBLK, H1], BF16)
            nc.vector.scalar_tensor_tensor(lr[:], tself[:], NEG, tself[:],
                                           OP.mult, OP.max)
            nc.scalar.activation(wself[:], lr[:], AF.Exp)
        if cfg.stop_after <= 1:
            nc.gpsimd.dma_start(y[0:128, :], wself[0:128, 0, 0:1])
            return

        # ---------------- layer 1 ----------------
        with (
            tc.tile_pool(name="l1g", bufs=3) as gpool,
            tc.tile_pool(name="l1s", bufs=3) as spool,
            tc.tile_pool(name="l1w", bufs=3) as wpool,
            tc.tile_pool(name="l1gw", bufs=10) as gwpool,
            tc.tile_pool(name="l1ng", bufs=2) as ngp,
            tc.tile_pool(name="l1ev", bufs=1) as evp,
            tc.tile_pool(name="ps_hs", bufs=4, space="PSUM") as pshs,
            tc.tile_pool(name="ps_ss", bufs=2, space="PSUM") as psss,
            tc.tile_pool(name="ps_blk", bufs=2, space="PSUM") as psblk,
        ):
            windows = {}

            def get_window(w):
                if w in windows:
                    return windows[w]
                n = min(WCH, NCH - w * WCH)
                xtw = gpool.tile([128, 1, WCH * 128], BF16, tag="xtw")
                # transposed gathers crash above 512 idxs -> 4-chunk pieces
                for q0 in range(0, n, 4):
                    qn = min(4, n - q0)
                    nc.gpsimd.dma_gather(
                        xtw[:, :, q0 * 128:(q0 + qn) * 128], ins["XCTAB"],
                        IDX1[:, (w * WCH + q0) * 8:(w * WCH + q0 + qn) * 8],
                        qn * 128, qn * 128, 128, transpose=True)
                Sb = spool.tile([128, WCH, 128], BF16, tag="Sb")
                nc.sync.dma_start(
                    Sb[:].rearrange("p c j -> p (c j)")[:, 0:n * 128],
                    ins["SH"][w * 128:(w + 1) * 128, 0:n * 128])
                STb = spool.tile([128, WCH, 128], BF16, tag="STb")
                nc.sync.dma_start(
                    STb[:].rearrange("p c j -> p (c j)")[:, 0:n * 128],
                    ins["STH"][w * 128:(w + 1) * 128, 0:n * 128])
                # window logits: ssrc (tiny matmul) + sdst expand, batched
                ss_ps = psss.tile([128, WCH, H1], F32, tag="ssps")
                for ci in range(n):
                    bb = st.wblocks[w][ci]
                    nc.tensor.matmul(ss_ps[:, ci, :],
                                     xtw[0:cfg.K1, 0, ci * 128:(ci + 1) * 128],
                                     A1SD[:, H1:2 * H1],
                                     start=(ci == 0), stop=False,
                                     skip_group_check=True)
                    nc.tensor.matmul(ss_ps[:, ci, :], STb[:, ci, :],
                                     sdss[:, bb, 0:H1],
                                     start=False, stop=(ci == n - 1),
                                     skip_group_check=True)
                ss_sb = wpool.tile([128, WCH, H1], F32, tag="sssb")
                nc.scalar.copy(ss_sb[:, 0:n, :], ss_ps[:, 0:n, :])
                lr = wpool.tile([128, WCH, H1], F32, tag="lr")
                nc.vector.scalar_tensor_tensor(lr[:, 0:n, :], ss_sb[:, 0:n, :],
                                               NEG, ss_sb[:, 0:n, :],
                                               OP.mult, OP.max)
                wv = wpool.tile([128, WCH, H1], BF16, tag="wv")
                nc.scalar.activation(wv[:, 0:n, :], lr[:, 0:n, :], AF.Exp)
                windows[w] = (xtw, Sb, wv)
                return windows[w]

            # item stream: per block a dense self pseudo-chunk then edge chunks
            items = []
            slot = 0
            for b in range(NBLK):
                nb = int(st.nch[b])
                items.append(("self", b, None, nb == 0))
                for kk in range(nb):
                    items.append(("edge", b, slot, kk == nb - 1))
                    slot += 1
            NIT = len(items)
            NWA = (NCH + WCH - 1) // WCH
            ng_of = {}

            def stash(b, blk_ps):
                if b % GRP == 0:
                    ng_of["num"] = ngp.tile([128, GRP, F1], BF16, tag="numbf",
                                            name="numbf")
                    ng_of["den"] = ngp.tile([128, GRP, H1], F32, tag="denf",
                                            name="denf")
                num_bf, den_f = ng_of["num"], ng_of["den"]
                nc.scalar.copy(num_bf[:, b % GRP, :], blk_ps[:, 0:F1])
                nc.vector.tensor_copy(den_f[:, b % GRP, :],
                                      blk_ps[:, F1:F1 + H1])
                if b % GRP == GRP - 1 or b == NBLK - 1:
                    g0 = (b // GRP) * GRP
                    gn = b - g0 + 1
                    rd = evp.tile([128, GRP, H1], F32, tag="rd")
                    nc.vector.reciprocal(rd[:, 0:gn, :], den_f[:, 0:gn, :])
                    rdb = evp.tile([128, GRP, H1], BF16, tag="rdb")
                    nc.vector.tensor_copy(rdb[:, 0:gn, :], rd[:, 0:gn, :])
                    xg = evp.tile([128, GRP, F1], BF16, tag="xg")
                    nc.vector.tensor_tensor(
                        xg[:, 0:gn, :].rearrange("p g (h d) -> p g h d", d=cfg.D1),
                        num_bf[:, 0:gn, :].rearrange("p g (h d) -> p g h d",
                                                     d=cfg.D1),
                        rdb[:, 0:gn, :].rearrange("p g (h u) -> p g h u", u=1)
                            .to_broadcast((128, gn, H1, cfg.D1)),
                        OP.mult)
                    # elu: exp(min(x,0)) - 1 + max(x,0)
                    tm = evp.tile([128, GRP, F1], BF16, tag="tm")
                    nc.vector.tensor_scalar(tm[:, 0:gn, :], xg[:, 0:gn, :],
                                            0.0, None, OP.min)
                    te = evp.tile([128, GRP, F1], BF16, tag="te")
                    nc.scalar.activation(te[:, 0:gn, :], tm[:, 0:gn, :], AF.Exp)
                    nc.vector.tensor_scalar(tm[:, 0:gn, :], xg[:, 0:gn, :],
                                            0.0, -1.0, OP.max, OP.add)
                    nc.vector.tensor_tensor(x2_all[:, g0:g0 + gn, :],
                                            te[:, 0:gn, :], tm[:, 0:gn, :],
                                            OP.add)

            blk_ps = [None]

            def l1_stage_a(i):
                """window ensure + hs matmul; returns consume-state."""
                kind, b, sl, last = items[i]
                if kind == "edge":
                    w, wi = divmod(sl, WCH)
                    xtw, Sb, wv = get_window(w)
                    if w + 1 < NWA:
                        get_window(w + 1)     # prefetch next window
                    lhsT = xtw[0:cfg.K1, 0, wi * 128:(wi + 1) * 128]
                    S = Sb[:, wi, :]
                    wvv = wv[:, wi, :]
                else:
                    lhsT = XTD[:, b * 128:(b + 1) * 128]
                    S = IDENT[:]
                    wvv = wself[:, b, :]
                hs_ps = pshs.tile([128, F1], F32, tag="hs")
                nc.tensor.matmul(hs_ps[:], lhsT, W1E[:], start=True, stop=True,
                                 skip_group_check=True)
                return (i, kind, b, last, S, wvv, hs_ps)

            def l1_stage_b(stt):
                i, kind, b, last, S, wvv, hs_ps = stt
                gw = gwpool.tile([128, F1], BF16, tag="gw")
                if i % 2 == 0:
                    src_ap = hs_ps[:]
                else:
                    hs_bf = gwpool.tile([128, F1], BF16, tag="hsbf")
                    nc.scalar.copy(hs_bf[:], hs_ps[:])
                    src_ap = hs_bf[:]
                nc.vector.tensor_tensor(
                    gw[:].rearrange("p (h d) -> p h d", d=cfg.D1),
                    src_ap.rearrange("p (h d) -> p h d", d=cfg.D1),
                    wvv.rearrange("p (h u) -> p h u", u=1)
                        .to_broadcast((128, H1, cfg.D1)),
                    OP.mult)
                if kind == "self":
                    blk_ps[0] = psblk.tile([128, F1 + H1], F32, tag="blk",
                                           name="blkps")
                bp = blk_ps[0]
                nc.tensor.matmul(bp[:, 0:F1], S, gw[:],
                                 start=(kind == "self"), stop=last,
                                 skip_group_check=True)
                nc.tensor.matmul(bp[:, F1:F1 + H1], S, wvv,
                                 start=False, stop=last,
                                 skip_group_check=True)
                if last:
                    stash(b, bp)

            pend = None
            for i in range(NIT):
                cur = l1_stage_a(i)
                if pend is not None:
                    l1_stage_b(pend)
                pend = cur
            l1_stage_b(pend)

        if cfg.stop_after <= 2:
            nc.gpsimd.dma_start(y[0:128, :], x2_all[0:128, 0, 0:1])
            return

        # ---------------- h2 build + AllGather + compaction ----------------
        F2E = F2 + 2 * H2
        with (
            tc.tile_pool(name="h2sb", bufs=2) as hsb,
            tc.tile_pool(name="ps_h2", bufs=2, space="PSUM") as psh,
            tc.tile_pool(name="ps_h2t", bufs=2, space="PSUM") as psht,
        ):
            for b in range(NBLK):
                rows = min(128, cfg.SHARD - b * 128)
                xt2_ps = psht.tile([128, 2, 128], BF16, tag="x2t")
                for k in range(2):
                    nc.tensor.transpose(xt2_ps[:, k, :],
                                        x2_all[:, b, k * 128:(k + 1) * 128],
                                        IDENT[:])
                xt2 = hsb.tile([128, 2, 128], BF16, tag="x2sb")
                nc.vector.tensor_copy(xt2[:], xt2_ps[:])
                h2_ps = psh.tile([128, F2E], F32, tag="h2ps")
                for k in range(2):
                    nc.tensor.matmul(h2_ps[:], xt2[:, k, :], W2F[:, k, :],
                                     start=(k == 0), stop=(k == 1),
                                     skip_group_check=True)
                nc.scalar.copy(h2_sb[:, b, :], h2_ps[:])
                nc.sync.dma_start(cc_in[b * 128:b * 128 + rows, 0:F2 + H2],
                                  h2_sb[0:rows, b, 0:F2 + H2])
            # self weights for layer 2
            t2 = hsb.tile([128, NBLK, H2], BF16, tag="t2")
            nc.vector.tensor_tensor(t2[:], h2_sb[:, :, F2:F2 + H2],
                                    h2_sb[:, :, F2 + H2:F2E], OP.add)
            lr2 = hsb.tile([128, NBLK, H2], BF16, tag="lr2")
            nc.vector.scalar_tensor_tensor(lr2[:], t2[:], NEG, t2[:],
                                           OP.mult, OP.max)
            nc.scalar.activation(w2self[:], lr2[:], AF.Exp)

        if cfg.stop_after <= 3:
            nc.gpsimd.dma_start(y[0:128, :], h2_sb[0:128, 0, 0:1])
            return

        if cfg.timing_single_core:
            nc.sync.dma_start(cc_out[0:cfg.SHARD, :], cc_in[:])
        else:
            nc.gpsimd.collective_compute(
                "AllGather", OP.bypass,
                replica_groups=[list(range(cfg.NC))],
                ins=[cc_in[:]],
                outs=[cc_out[:]],
            )

        if cfg.stop_after <= 4:
            nc.gpsimd.dma_start(y[0:128, :], h2_sb[0:128, 0, 0:1])
            return

        # compaction: gather unique rows from cc_out into local compact ctab
        with tc.tile_pool(name="cg", bufs=4) as cgp:
            ctv = ctab.rearrange("(c p) f -> p c f", p=128)
            F2U = F2 + H2    # used cols of a cc row
            for sec, base, ncc in ((0, 0, st.KLOC), (1, st.KLOC, st.KHIC)):
                tab = cc_out[0:cfg.SPLIT, :] if sec == 0 else \
                    cc_out[cfg.SPLIT:cfg.N, :]
                for w0 in range(0, ncc, WCH):
                    n = min(WCH, ncc - w0)
                    gt = cgp.tile([128, WCH, CTC], BF16, tag="cgt")
                    for q0 in range(0, n, 8):  # <=1024 idxs per gather
                        qn = min(8, n - q0)
                        nc.gpsimd.dma_gather(
                            gt[:, q0:q0 + qn, :], tab,
                            IDXC[:, (base + w0 + q0) * 8:
                                 (base + w0 + q0 + qn) * 8],
                            qn * 128, qn * 128, CTC)
                    nc.sync.dma_start(
                        ctv[:, base + w0:base + w0 + n, 0:F2U],
                        gt[:, 0:n, 0:F2U])

        if cfg.stop_after <= 5:
            nc.gpsimd.dma_start(y[0:128, :], h2_sb[0:128, 0, 0:1])
            return

        # ---------------- layer 2 ----------------
        with (
            tc.tile_pool(name="l2g", bufs=3) as gpool,
            tc.tile_pool(name="l2s", bufs=3) as spool,
            tc.tile_pool(name="l2w", bufs=3) as wpool,
            tc.tile_pool(name="l2gw", bufs=10) as gwpool,
            tc.tile_pool(name="l2ng", bufs=2) as ngp,
            tc.tile_pool(name="l2ev", bufs=1) as evp,
            tc.tile_pool(name="ps_ss2", bufs=3, space="PSUM") as psss,
            tc.tile_pool(name="ps_blk2", bufs=2, space="PSUM") as psblk,
        ):
            windows = {}

            def get_window2(w):
                if w in windows:
                    return windows[w]
                n = min(WCH, NCH - w * WCH)
                gt = gpool.tile([128, WCH, CTC], BF16, tag="gt")
                for q0 in range(0, n, 8):  # <=1024 idxs per gather
                    qn = min(8, n - q0)
                    nc.gpsimd.dma_gather(
                        gt[:, q0:q0 + qn, :], ctab,
                        IDX1[:, (w * WCH + q0) * 8:(w * WCH + q0 + qn) * 8],
                        qn * 128, qn * 128, CTC)
                Sb = spool.tile([128, WCH, 128], BF16, tag="Sb2")
                nc.sync.dma_start(
                    Sb[:].rearrange("p c j -> p (c j)")[:, 0:n * 128],
                    ins["SH"][w * 128:(w + 1) * 128, 0:n * 128])
                STb = spool.tile([128, WCH, 128], BF16, tag="STb2")
                nc.sync.dma_start(
                    STb[:].rearrange("p c j -> p (c j)")[:, 0:n * 128],
                    ins["STH"][w * 128:(w + 1) * 128, 0:n * 128])
                ss_ps = psss.tile([128, WCH, H2], F32, tag="ssps2")
                for ci in range(n):
                    bb = st.wblocks[w][ci]
                    nc.tensor.matmul(ss_ps[:, ci, :], STb[:, ci, :],
                                     h2_sb[:, bb, F2 + H2:F2E],
                                     start=(ci == 0), stop=(ci == n - 1),
                                     skip_group_check=True)
                t = wpool.tile([128, WCH, H2], F32, tag="t")
                nc.vector.tensor_tensor(t[:, 0:n, :], ss_ps[:, 0:n, :],
                                        gt[:, 0:n, F2:F2 + H2], OP.add)
                lr = wpool.tile([128, WCH, H2], F32, tag="lr")
                nc.vector.scalar_tensor_tensor(lr[:, 0:n, :], t[:, 0:n, :],
                                               NEG, t[:, 0:n, :],
                                               OP.mult, OP.max)
                wv = wpool.tile([128, WCH, H2], BF16, tag="wv")
                nc.scalar.activation(wv[:, 0:n, :], lr[:, 0:n, :], AF.Exp)
                windows[w] = (gt, Sb, wv)
                return windows[w]

            items = []
            slot = 0
            for b in range(NBLK):
                nb = int(st.nch[b])
                items.append(("self", b, None, nb == 0))
                for kk in range(nb):
                    items.append(("edge", b, slot, kk == nb - 1))
                    slot += 1
            NIT = len(items)
            NWA = (NCH + WCH - 1) // WCH
            ng_of = {}

            def stash2(b, blk_ps):
                if b % GRP == 0:
                    ng_of["num"] = ngp.tile([128, GRP, F2], BF16, tag="numbf2",
                                            name="numbf2")
                    ng_of["den"] = ngp.tile([128, GRP, H2], F32, tag="denf2",
                                            name="denf2")
                num_bf, den_f = ng_of["num"], ng_of["den"]
                nc.scalar.copy(num_bf[:, b % GRP, :], blk_ps[:, 0:F2])
                nc.vector.tensor_copy(den_f[:, b % GRP, :],
                                      blk_ps[:, F2:F2 + H2])
                if b % GRP == GRP - 1 or b == NBLK - 1:
                    g0 = (b // GRP) * GRP
                    gn = b - g0 + 1
                    rd = evp.tile([128, GRP, H2], F32, tag="rd2")
                    nc.vector.reciprocal(rd[:, 0:gn, :], den_f[:, 0:gn, :])
                    rdb = evp.tile([128, GRP, H2], BF16, tag="rdb2")
                    nc.vector.tensor_copy(rdb[:, 0:gn, :], rd[:, 0:gn, :])
                    xg = evp.tile([128, GRP, F2], BF16, tag="xg2")
                    nc.vector.tensor_tensor(
                        xg[:, 0:gn, :].rearrange("p g (h d) -> p g h d", d=cfg.D2),
                        num_bf[:, 0:gn, :].rearrange("p g (h d) -> p g h d",
                                                     d=cfg.D2),
                        rdb[:, 0:gn, :].rearrange("p g (h u) -> p g h u", u=1)
                            .to_broadcast((128, gn, H2, cfg.D2)),
                        OP.mult)
                    tm = evp.tile([128, GRP, F2], BF16, tag="tm2")
                    nc.vector.tensor_scalar(tm[:, 0:gn, :], xg[:, 0:gn, :],
                                            0.0, None, OP.min)
                    te = evp.tile([128, GRP, F2], BF16, tag="te2")
                    nc.scalar.activation(te[:, 0:gn, :], tm[:, 0:gn, :], AF.Exp)
                    nc.vector.tensor_scalar(tm[:, 0:gn, :], xg[:, 0:gn, :],
                                            0.0, -1.0, OP.max, OP.add)
                    fc = evp.tile([128, GRP, F2], BF16, tag="fc")
                    nc.vector.tensor_tensor(fc[:, 0:gn, :], te[:, 0:gn, :],
                                            tm[:, 0:gn, :], OP.add)
                    nc.vector.tensor_tensor(
                        fc[:, 0:gn, :], fc[:, 0:gn, :],
                        WFC[:].rearrange("p (u f) -> p u f", u=1)
                            .to_broadcast((128, gn, F2)),
                        OP.mult)
                    red = evp.tile([128, GRP], F32, tag="red")
                    nc.vector.tensor_reduce(
                        red[:, 0:gn].rearrange("p (g u) -> p g u", u=1),
                        fc[:, 0:gn, :], mybir.AxisListType.X, OP.add)
                    ys = evp.tile([128, GRP], F32, tag="ys")
                    nc.scalar.activation(ys[:, 0:gn], red[:, 0:gn], AF.Sigmoid,
                                         bias=BFCC[:, 0:1])
                    for j in range(gn):
                        bb = g0 + j
                        rws = min(128, cfg.SHARD - bb * 128)
                        nc.sync.dma_start(y[bb * 128:bb * 128 + rws, :],
                                          ys[0:rws, j:j + 1])

            blk_ps = None
            for i in range(NIT):
                kind, b, sl, last = items[i]
                if kind == "edge":
                    w, wi = divmod(sl, WCH)
                    gt, Sb, wv = get_window2(w)
                    if w + 1 < NWA:
                        get_window2(w + 1)    # prefetch next window
                    src_ap = gt[:, wi, 0:F2]
                    S = Sb[:, wi, :]
                    wvv = wv[:, wi, :]
                else:
                    src_ap = h2_sb[:, b, 0:F2]
                    S = IDENT[:]
                    wvv = w2self[:, b, :]
                gw = gwpool.tile([128, F2], BF16, tag="gw2")
                nc.vector.tensor_tensor(
                    gw[:].rearrange("p (h d) -> p h d", d=cfg.D2),
                    src_ap.rearrange("p (h d) -> p h d", d=cfg.D2),
                    wvv.rearrange("p (h u) -> p h u", u=1)
                        .to_broadcast((128, H2, cfg.D2)),
                    OP.mult)
                if kind == "self":
                    blk_ps = psblk.tile([128, F2 + H2], F32, tag="blk2")
                nc.tensor.matmul(blk_ps[:, 0:F2], S, gw[:],
                                 start=(kind == "self"), stop=last,
                                 skip_group_check=True)
                nc.tensor.matmul(blk_ps[:, F2:F2 + H2], S, wvv,
                                 start=False, stop=last,
                                 skip_group_check=True)
                if last:
                    stash2(b, blk_ps)


# --------------------------------------------------------------------------
#  host entry
# --------------------------------------------------------------------------

def build(inputs, cfg: Cfg):
    ei = np.asarray(inputs["edge_index"])
    src, dst = ei[0], ei[1]
    st = prep_edges(cfg, src, dst)
    in_maps = host_inputs(cfg, st, inputs)

    nc = bacc.Bacc("TRN2", target_bir_lowering=False, debug=False,
                   num_devices=cfg.NC, dynamic_dma_scratch_size=65536)
    ins_aps = {}
    for k, v in in_maps[0].items():
        dt = mybir.dt.from_np(v.dtype)
        ins_aps[k] = nc.dram_tensor(k, list(v.shape), dt,
                                    kind="ExternalInput").ap()
    y_ap = nc.dram_tensor("y", [cfg.NBLK * 128, 1], F32,
                          kind="ExternalOutput").ap()

    with tile.TileContext(nc) as tc:
        emit_gat(tc, {"y": y_ap}, ins_aps, cfg, st)
    nc.compile()
    return nc, in_maps, st


def build_and_run(inputs, cfg: Cfg, trace=False):
    nc, in_maps, st = build(inputs, cfg)
    res = run_bass_kernel_spmd(nc, in_maps, core_ids=list(range(cfg.NC)),
                               trace=trace)
    parts = [res.results[c]["y"][:min(cfg.SHARD, cfg.N - c * cfg.SHARD)]
             for c in range(cfg.NC)]
    out = np.concatenate(parts, axis=0)
    return out, res


def kernel(**inputs):
    cfg = Cfg()
    out, _ = build_and_run(inputs, cfg)
    return out.astype(np.float32)
